# revision 75
# baseline (speedup 1.0000x reference)
"""Multi-head attention (B=2, S=2048, D=1024, H=16, causal) on 8 Trainium2 cores.

Sharding: core c handles batch b = c // 4 and head group g = c % 4 (4 heads,
d_model column slice [256*g, 256*g+256)).  QKV projections are computed per
core against the full sequence of its batch; attention runs per head in a
"scores-transposed" [k, q] layout; the output projection produces a per-core
partial [S, D] (bf16) that the host sums over the 4 head-group cores.

v5 perf structure (PE-bound; ~160us on 8 cores):
- x is transposed AND 512-col-blocked on the HOST to [NSLAB, 128, DK*SB]:
  each block is ONE fully-contiguous [128 x 8KB-row] DMA.  DMA triggers
  cost ~730ns of serial queue time each regardless of size, and the DMA
  engines split HBM bandwidth fairly per QUEUE — so ALL input loads go on
  the sync queue as few big triggers in exact need order (arrival order ==
  need order at full 358 GB/s).
- Scores use d-major PSUM tiles (one [P,2,SB] tile per k-tile of a pair,
  banks = heads) with exp per d over both heads: the two 64-row score MMs
  of a d become READY together, so the Tile scheduler keeps them adjacent
  and they run CONCURRENT in alternating PE row groups (h0/h64) — measured
  ~82% of adjacent score MMs start within 6ns.  (Per-hh tiles stagger
  readiness by the previous iteration's exps and the scheduler regroups MMs
  per row group: no concurrency.  One merged 4-bank tile forces scores(i+1)
  to wait the full exp(i): 1.5us PE stalls + HAM re-throttle.)
- P@V runs in fp8e4 DoubleRow perf mode: expT ([P, 2, SB]) and v
  ([P, 2, HC, P]) are pair-indexed along k-tiles; each k-tile PAIR is one
  fused virtual-K=256 matmul at 2x bf16 throughput.  exp writes fp8 directly
  (bias -3.5: device fp8e4 is IEEE e4m3, max 240 WITH an inf encoding; max
  raw causal score ~68 -> exp arg 68/8-3.5=5.0 -> e^5=148 < 240.  At the
  original bias -3 the max exp was 237 — one fp8 step from inf, and an inf
  exp makes outp/denominator inf -> NaN output rows).  The softmax
  denominator comes from 64 ones-columns appended to V.
- The V projection for s-tiles 4-15 runs fp8 DoubleRow (x and Wv fp8 from
  host, o-chunk pairs fused); tiles 0-3 stay bf16 — they feed the all-bf16
  slab-0 attention (fp8 error concentrates in early rows where softmax
  averages few values).  Q/K projections MUST stay bf16: fp8 k alone costs
  ~2.3e-2 rel err (score noise hits early-row softmax hard) vs the 2e-2
  gate.
- Slabs run in DESCENDING order (3,2,1,0); projection / output-projection
  units are WOVEN into the attention iterations as PE filler.  The pending
  P@V is flushed BEFORE each iteration's fillers (it is always data-ready;
  a filler stuck on a DMA arrival would block it in the PE FIFO).
  Keep-alive matmuls (no data deps) bridge DMA-arrival gaps in the lead-in
  and at iteration starts so the HAM clock gate never sees a full idle
  window (which would re-throttle the PE to 1.2 GHz).
- Final 4 output-projection units draw PSUM from the post-exp-idle sps pool
  (4-deep pipelining with pp) and store per 512-half for an earlier finish.
"""

import functools
import os
import numpy as np
import ml_dtypes

import concourse.bass as bass
import concourse.bacc as bacc
import concourse.tile as tile
import concourse.mybir as mybir
from concourse.bass_utils import run_bass_kernel_spmd

dt = mybir.dt
F32 = dt.float32
BF16 = dt.bfloat16
FP8 = dt.float8e4
AFT = mybir.ActivationFunctionType

B, S, D = 2, 2048, 1024
H, DH = 16, 64
NCORES = 8
GROUPS = NCORES // B            # 4 head-groups
HC = H // GROUPS                # 4 heads per core
C = HC * DH                     # 256 = per-core head-column slice
P = 128
DK = D // P                     # 8 d_in chunks
SB = 512                        # q-slab width
NSLAB = S // SB                 # 4
KT = S // P                     # 16 k tiles
SCALE = 1.0 / float(np.sqrt(DH))


def _build(mask_mode: str):
    """mask_mode: 'causal' | 'none' | 'generic'. Returns compiled Bacc."""
    assert mask_mode in ("causal", "none", "generic")
    causal = mask_mode == "causal"
    nc = bacc.Bacc("TRN2", target_bir_lowering=False, debug=False)

    # host-transposed x, 512-col blocked: xT[b, p, o, c] = x[512*b + c, 128*o + p]
    # -> each 512-block is ONE fully-contiguous [128 x 4096] DMA (one trigger).
    xq_d = nc.dram_tensor("xq", [NSLAB, P, DK * SB], BF16, kind="ExternalInput").ap()
    xk_d = nc.dram_tensor("xk", [NSLAB, P, DK * SB], BF16, kind="ExternalInput").ap()
    # xv block 0 stays bf16 (feeds the all-bf16 slab-0 v); blocks 1-3 are fp8
    # so their vproj runs DoubleRow (v is quantized to fp8 for P@V anyway)
    xv0_d = nc.dram_tensor("xv0", [P, DK * SB], BF16, kind="ExternalInput").ap()
    xv8_d = nc.dram_tensor("xv8", [NSLAB - 1, P, DK * SB], FP8,
                           kind="ExternalInput").ap()
    # all bf16 constants packed into one tensor: wq|wk|wv|wo|strips
    cb_d = nc.dram_tensor("cb", [P, 5 * 2048], BF16, kind="ExternalInput").ap()
    cb8_d = nc.dram_tensor("cb8", [P, 2048], FP8, kind="ExternalInput").ap()
    cf_d = nc.dram_tensor("cf", [P, 260], F32, kind="ExternalInput").ap()
    if mask_mode == "generic":
        maskT_d = nc.dram_tensor("maskT", [S, S], BF16, kind="ExternalInput").ap()
    o_d = nc.dram_tensor("o", [S, D], BF16, kind="ExternalOutput").ap()
    dump = os.environ.get("K_DUMP") == "1"
    if dump:
        qT_o = nc.dram_tensor("qT_o", [P, C // P, S], BF16, kind="ExternalOutput").ap()
        kT_o = nc.dram_tensor("kT_o", [P, C // P, S], BF16, kind="ExternalOutput").ap()
        v_o = nc.dram_tensor("v_o", [P, KT // 2, 2, HC, P], FP8, kind="ExternalOutput").ap()
        vbf_o = nc.dram_tensor("vbf_o", [P, 4, HC, P], BF16, kind="ExternalOutput").ap()
        heads_o = nc.dram_tensor("heads_o", [P, C // P, S], BF16, kind="ExternalOutput").ap()
        strips_o = nc.dram_tensor("strips_o", [P, SB // P, SB], BF16, kind="ExternalOutput").ap()
        expT_o = nc.dram_tensor("expT_o", [P, 2, 16, 2, SB], FP8, kind="ExternalOutput").ap()
        spc_o = nc.dram_tensor("spc_o", [P, 8, 2, SB], F32, kind="ExternalOutput").ap()
        outp_o = nc.dram_tensor("outp_o", [P, 2, 2, SB], F32, kind="ExternalOutput").ap()
        ob_o = nc.dram_tensor("ob_o", [P, D], BF16, kind="ExternalOutput").ap()

    with tile.TileContext(nc) as tc:
        with (
            tc.tile_pool(name="consts", bufs=1) as consts,
            tc.tile_pool(name="xT", bufs=3) as xT_pool,
            tc.tile_pool(name="acts", bufs=1) as acts,
            tc.tile_pool(name="expT", bufs=4) as exp_pool,
            tc.tile_pool(name="stage", bufs=2) as stage,
            tc.tile_pool(name="pp", bufs=2, space="PSUM") as pp,
            tc.tile_pool(name="sps", bufs=2, space="PSUM") as sps,
            tc.tile_pool(name="otp", bufs=2, space="PSUM") as otp,
        ):
            # ---- constants: one dedicated tile per constant, each written by
            # exactly one whole-tile DMA (readers then depend on the whole
            # write — no sub-range matching through view slices).
            wk_t = consts.tile([P, DK, C], BF16, name="wk")
            wq_t = consts.tile([P, DK, C], BF16, name="wq")
            wv_t = consts.tile([P, DK, C], BF16, name="wv")
            wv8_t = consts.tile([P, DK, C], FP8, name="wv8")
            wo_t = consts.tile([P, C // P, D], BF16, name="wo")
            strips_t = consts.tile([P, SB // P, SB], BF16, name="strips")
            cf_sb = consts.tile([P, 260], F32)
            # weights go FIRST on the sync HWDGE ring (SWDGE is slow to
            # start and wk gates the very first projection)
            wq_sb, wk_sb, wv_sb = wq_t[:], wk_t[:], wv_t[:]
            wv8_sb = wv8_t[:]
            wo_sb, strips_sb = wo_t[:], strips_t[:]
            bq_sb = cf_sb[:, 0:2]
            bk_sb = cf_sb[:, 2:4]
            bvb_sb = cf_sb[:, 4:260]
            # warmup source: the very first DVE op so the PE warmup
            # matmuls can start as soon as the engines come up
            wsrc = stage.tile([P, SB], BF16, tag="warm", name="wsrc")
            nc.vector.memset(wsrc[:], 0.5)
            # bias -3.5: max exp e^5.0 = 148 < fp8e4 max 240 (see module
            # docstring — bias -3 sat one fp8 step below the inf encoding).
            # NOT lower: typical exp values must stay in fp8 NORMAL range
            # (>= 2^-6); bias -4 pushed them subnormal (rel err 6e-3 -> 2e-2).
            expbias_sb = consts.tile([P, 1], F32)
            nc.vector.memset(expbias_sb[:], -3.5)

            # ---- x loads.  Each DMA trigger costs ~730ns of serial queue
            # time regardless of size, so x moves as ONE contiguous trigger
            # per 512-col block (the consumption granule), ALL on the sync
            # queue in exact need order (see module docstring: per-queue
            # fair-share bandwidth makes a second queue counterproductive).
            xqT = xT_pool.tile([P, NSLAB, DK, SB], BF16, tag="xT", name="xqT")
            xkT = xT_pool.tile([P, NSLAB, DK, SB], BF16, tag="xT", name="xkT")
            xv0T = xT_pool.tile([P, DK, SB], BF16, tag="xT", name="xv0T")
            xv8T = xT_pool.tile([P, NSLAB - 1, DK, SB], FP8, tag="xT",
                                name="xv8T")

            def ld(eng, xt, x_d, b):
                eng.dma_start(xt[:, b], x_d[b])

            # ONE queue, exact need order: the DMA engines split HBM
            # bandwidth fairly per QUEUE, so a second queue running early
            # non-critical transfers steals bandwidth from the critical
            # stream.  Within a queue, transfers complete in order at full
            # rate — need-order IS arrival-order.
            nc.sync.dma_start(wk_t[:], cb_d[:, 2048:4096])
            ld(nc.sync, xkT, xk_d, 0)
            nc.sync.dma_start(wq_t[:], cb_d[:, 0:2048])
            nc.sync.dma_start(cf_sb[:], cf_d)
            ld(nc.sync, xqT, xq_d, 3)
            nc.sync.dma_start(wv_t[:], cb_d[:, 4096:6144])
            nc.sync.dma_start(xv0T[:], xv0_d)
            nc.sync.dma_start(wv8_t[:], cb8_d)
            ld(nc.sync, xkT, xk_d, 1)
            nc.sync.dma_start(xv8T[:, 0], xv8_d[0])
            nc.sync.dma_start(strips_t[:], cb_d[:, 8192:10240])
            ld(nc.sync, xkT, xk_d, 2)
            nc.sync.dma_start(xv8T[:, 1], xv8_d[1])
            ld(nc.sync, xkT, xk_d, 3)
            nc.sync.dma_start(xv8T[:, 2], xv8_d[2])
            nc.sync.dma_start(wo_t[:], cb_d[:, 6144:8192])
            ld(nc.sync, xqT, xq_d, 2)
            ld(nc.sync, xqT, xq_d, 1)
            ld(nc.sync, xqT, xq_d, 0)

            qT_sb = acts.tile([P, C // P, S], BF16)       # [d_out, s] head-major
            kT_sb = acts.tile([P, C // P, S], BF16)
            headsT_sb = acts.tile([P, C // P, S], BF16)
            # v pair-indexed fp8: [p, kpair, h, i, col]; cols 0:64 v, 64:128
            # ones.  The h-major layout makes the DoubleRow lhsT slice
            # [:, kp, h, :, :] a contiguous trailing block (dependency
            # tracking of the interleaved [:, kp, :, h, :] form missed the
            # vproj writes -> first-execution read-before-write NaN).
            v_sb = acts.tile([P, KT // 2, HC, 2, P], FP8)
            nc.gpsimd.memset(v_sb[:, :, :, :, DH:P], 1.0)
            # bf16 copy of v k-tiles 0..3 for the all-bf16 slab-0 attention
            vbf_sb = acts.tile([P, 4, HC, P], BF16)
            nc.gpsimd.memset(vbf_sb[:, :, :, DH:P], 1.0)

            def proj_unit(which, co, j):
                ps = pp.tile([P, SB], F32, tag="pp", name="proj_ps")
                if which == "q":
                    b_sb, outT = bq_sb, qT_sb
                    for o in range(DK):
                        nc.tensor.matmul(
                            ps[:],
                            lhsT=wq_sb[:, o, co * P:(co + 1) * P],
                            rhs=xqT[:, j, o, :],
                            start=(o == 0), stop=(o == DK - 1))
                else:
                    b_sb, outT = bk_sb, kT_sb
                    for o in range(DK):
                        nc.tensor.matmul(
                            ps[:],
                            lhsT=wk_sb[:, o, co * P:(co + 1) * P],
                            rhs=xkT[:, j, o, :],
                            start=(o == 0), stop=(o == DK - 1))
                nc.vector.tensor_scalar_add(
                    outT[:, co, j * SB:(j + 1) * SB], ps[:], b_sb[:, co:co + 1])

            def vproj_unit(st0, nst=2):
                for st in range(st0, st0 + nst):
                    ps = pp.tile([P, SB], F32, tag="pp", name="vproj_ps")
                    c0, c1 = (st % 4) * P, (st % 4 + 1) * P
                    if st < 4:
                        for o in range(DK):
                            nc.tensor.matmul(
                                ps[:, 0:C],
                                lhsT=xv0T[:, o, c0:c1],
                                rhs=wv_sb[:, o, :],
                                start=(o == 0), stop=(o == DK - 1))
                    else:
                        # fp8 x + fp8 w, o-chunk pairs fused via DoubleRow
                        for op in range(DK // 2):
                            nc.tensor.matmul(
                                ps[:, 0:C],
                                lhsT=xv8T[:, st // 4 - 1,
                                          2 * op:2 * op + 2, c0:c1],
                                rhs=wv8_sb[:, 2 * op:2 * op + 2, :],
                                start=(op == 0), stop=(op == DK // 2 - 1),
                                perf_mode=mybir.MatmulPerfMode.DoubleRow)
                    # one contiguous DVE add into a staging tile, then per-
                    # head gpsimd copies into v.  Per-head contiguous writes
                    # are required: a single strided (h-step 256) write is
                    # NOT seen as overlapping the DoubleRow lhsT read by the
                    # dependency tracker (first-exec read-before-write NaN);
                    # gpsimd does them so the DVE isn't op-count-bound.
                    vstage = stage.tile([P, C], BF16, tag="vst", name="vst")
                    nc.vector.tensor_add(vstage[:], ps[:, 0:C], bvb_sb[:])
                    for h in range(HC):
                        nc.gpsimd.tensor_copy(
                            v_sb[:, st // 2, h, st % 2, 0:DH],
                            vstage[:, h * DH:(h + 1) * DH])
                        if st < 4:
                            nc.gpsimd.tensor_copy(
                                vbf_sb[:, st, h, 0:DH],
                                vstage[:, h * DH:(h + 1) * DH])

            def oproj_unit(st, pool=None):
                ob = stage.tile([P, D], BF16, tag="ob", name="ob")
                if pool is None:
                    ps = {n2: pp.tile([P, SB], F32, tag="pp", name="o_ps")
                          for n2 in range(D // SB)}
                else:
                    # final units draw PSUM from the sps pool (free after the
                    # last exp): with pp's 2 bufs ALSO free this pipelines
                    # deeper instead of stalling on each unit's drains
                    pst = pool.tile([P, 2, SB], F32, tag="sps", name="o_ps2")
                    ps = {n2: pst[:, n2, :] for n2 in range(D // SB)}
                # cc-outer: both n2 matmuls of a cc share the stationary
                for cc in range(C // P):
                    for n2 in range(D // SB):
                        nc.tensor.matmul(
                            ps[n2][:],
                            lhsT=headsT_sb[:, cc, st * P:(st + 1) * P],
                            rhs=wo_sb[:, cc, n2 * SB:(n2 + 1) * SB],
                            start=(cc == 0), stop=(cc == C // P - 1))
                for n2 in range(D // SB):
                    # tail slabs (st<8 run during/after slab0): ACT is idle
                    # after the last exp while DVE still has normalize work —
                    # split the two PSUM->SBUF drains across both engines.
                    if st < 8 and n2 == 0:
                        nc.scalar.copy(ob[:, n2 * SB:(n2 + 1) * SB], ps[n2][:])
                    else:
                        nc.vector.tensor_copy(ob[:, n2 * SB:(n2 + 1) * SB],
                                              ps[n2][:])
                # stores go on the sync queue (idle once the input loads are
                # issued); a store trigger on the scalar queue would block
                # subsequent exp instructions — strict FIFO.  The final units
                # (sps pool) store per half so the first half streams out
                # while the second is still copying.
                if pool is None:
                    nc.sync.dma_start(o_d[st * P:(st + 1) * P, :], ob[:])
                else:
                    for n2 in range(D // SB):
                        nc.sync.dma_start(
                            o_d[st * P:(st + 1) * P, n2 * SB:(n2 + 1) * SB],
                            ob[:, n2 * SB:(n2 + 1) * SB])
                if dump and st == 12:
                    nc.sync.dma_start(ob_o, ob[:])

            def attn_slab(j, fillers=None):
                # fillers: {pair_index: [closures]}.  Tile derives
                # dependencies from EMISSION order (a reader only waits on
                # earlier-emitted writers), so every filler must be emitted
                # before its first consumer — never after it.
                fillers = dict(fillers or {})
                n_kt = 4 * (j + 1) if causal else KT
                bfslab = causal and j == 0    # slab 0 all-bf16
                edt = BF16 if bfslab else FP8
                etag = "expTbf" if bfslab else "expT"
                it = 0
                for hc in range(HC // 2):          # head pair (2hc, 2hc+1)
                    outp = [otp.tile([P, SB], F32, tag="otp", name=f"outp{hh}")
                            for hh in range(2)]
                    pend = [None, None]

                    def qlo(t):   # valid-q offset within slab for k-tile t
                        if not causal:
                            return 0
                        return max(0, P * t - SB * j)

                    def make_pav(tb, qp, hh, expT_t):
                        # expT_t is the iteration's [P, 2(d), 2(hh), SB]
                        # tile; head hh's k-tile pair is the strided slice
                        # [:, :, hh, :]
                        h = 2 * hc + hh

                        def pav():
                            if bfslab:
                                for d_ in range(2):
                                    t = tb + d_
                                    ql = qlo(t)
                                    nc.tensor.matmul(
                                        outp[hh][:, ql:],
                                        lhsT=vbf_sb[:, t, h, :],
                                        rhs=expT_t[:, d_, hh, ql:],
                                        start=(t == 0), stop=(t == n_kt - 1))
                            elif (causal and tb >= 4 * j) or \
                                    mask_mode == "generic":
                                # masked pairs run as 2 plain fp8 matmuls:
                                # their [d_, hh, qp:] rhs reads structurally
                                # match the strip/mask tensor_mul writes, so
                                # the dependency tracker sees them
                                for d_ in range(2):
                                    nc.tensor.matmul(
                                        outp[hh][:, qp:],
                                        lhsT=v_sb[:, tb // 2, h, d_, :],
                                        rhs=expT_t[:, d_, hh, qp:],
                                        start=(tb == 0 and d_ == 0),
                                        stop=(d_ == 1 and tb == n_kt - 2))
                            else:
                                # fused P@V: one DoubleRow matmul per k-tile
                                # pair (2 fp8 weights/cell; virtual K=256)
                                nc.tensor.matmul(
                                    outp[hh][:, qp:],
                                    lhsT=v_sb[:, tb // 2, h, :, :],
                                    rhs=expT_t[:, :, hh, qp:],
                                    start=(tb == 0), stop=(tb == n_kt - 2),
                                    perf_mode=mybir.MatmulPerfMode.DoubleRow)
                        return pav

                    # d-major iteration: one [P,2,SB] PSUM tile per k-tile of
                    # the pair, each holding BOTH heads' scores (bank=hh).
                    # The 2 MMs of a d become READY together (single WAR edge
                    # on that d's exp), so the scheduler keeps them adjacent
                    # and — being different row groups (h0/h64) — they run
                    # CONCURRENT in the array.  (With per-hh tiles, readiness
                    # was staggered by the previous iteration's exps and the
                    # scheduler regrouped the MMs per row group — no
                    # concurrency.  With one merged 4-bank tile, scores(i+1)
                    # had to wait the FULL exp(i) — 1.5us PE stalls and HAM
                    # re-throttle.)  exp runs per d over both heads (N=1024);
                    # expT is [P, 2(d), 2(hh), SB] so the DR P@V rhs is the
                    # clean strided slice [:, :, hh, qp:].
                    for tb in range(0, n_kt, 2):
                        myfill = fillers.pop(it, ())
                        it += 1
                        qb = qlo(tb)
                        qp = qb
                        mloads = []
                        # both tiles write from the PAIR base qb: exp reads
                        # [qb:], and a never-written stale PSUM value there
                        # can exp() to inf -> fp8 INF -> NaN via the
                        # strip-zero multiply.
                        spd = [sps.tile([P, 2, SB], F32, tag="sps", name="sp")
                               for _ in range(2)]
                        expT4 = exp_pool.tile([P, 2, 2, SB], edt, tag=etag,
                                              name="expT")
                        for d_ in range(2):
                            t = tb + d_
                            for hh in range(2):
                                hp = DH * hh
                                nc.tensor.matmul(
                                    spd[d_][:, hh, qb:],
                                    lhsT=kT_sb[hp:hp + DH, hc,
                                               t * P:(t + 1) * P],
                                    rhs=qT_sb[hp:hp + DH, hc,
                                              j * SB + qb:(j + 1) * SB],
                                    start=True, stop=True,
                                    tile_position=(hp, 0))
                        # bias -3.5: keeps exp within fp8e4 range (softmax
                        # is shift-invariant; numerator and denominator
                        # share the e^-3.5 factor)
                        for d_ in range(2):
                            nc.scalar.activation(
                                expT4[:, d_, :, qb:],
                                spd[d_][:, 0:2, qb:], AFT.Exp, scale=SCALE,
                                bias=expbias_sb[:])
                        if dump and j == 3 and hc == 0 and not bfslab:
                            for hh in range(2):
                                nc.sync.dma_start(
                                    expT_o[:, hh, tb // 2, :, :],
                                    expT4[:, :, hh, :])
                        # the pending P@V (previous pair's) goes BEFORE the
                        # fillers: it is always data-ready (its v tiles came
                        # from the previous iteration), while a filler stuck
                        # on a DMA arrival would block it in the PE FIFO and
                        # idle the array long enough to re-throttle the clock
                        for hh in range(2):
                            if pend[hh] is not None:
                                pend[hh]()
                                pend[hh] = None
                        # fillers after this pair's scores (so they are not
                        # queued ahead of them) but before the P@V that may
                        # consume their output
                        for f in myfill:
                            f()
                        for d_ in range(2):
                            t = tb + d_
                            if mask_mode == "generic":
                                m_sb = stage.tile([P, SB], BF16, tag="msk",
                                                  name="m_sb")
                                nc.gpsimd.dma_start(
                                    m_sb[:],
                                    maskT_d[t * P:(t + 1) * P,
                                            j * SB:(j + 1) * SB])
                                mloads.append(m_sb)
                            if mask_mode == "generic":
                                for hh in range(2):
                                    nc.vector.tensor_mul(
                                        expT4[:, d_, hh, :],
                                        expT4[:, d_, hh, :],
                                        mloads[d_][:])
                            elif causal and t >= 4 * j:
                                # mask relative to the PAIR's q window:
                                # d_=0: triangle on 128 cols; d_=1: zero
                                # prefix + triangle over 256 cols; one mul
                                # covers BOTH heads' banks of this d
                                w = min((d_ + 1) * P, SB - qp)
                                for hh in range(2):
                                    nc.vector.tensor_mul(
                                        expT4[:, d_, hh, qp:qp + w],
                                        expT4[:, d_, hh, qp:qp + w],
                                        strips_sb[:, d_, 0:w])
                        for hh in range(2):
                            if pend[hh] is not None:
                                pend[hh]()
                            pend[hh] = make_pav(tb, qp, hh, expT4)
                    for hh in range(2):
                        pend[hh]()
                    if dump and j == 3 and hc == 0:
                        for hh in range(2):
                            oc = stage.tile([P, SB], F32, tag="recip",
                                            name="oc_dump")
                            nc.vector.tensor_copy(oc[:], outp[hh][:])
                            nc.sync.dma_start(outp_o[:, hh, 0, :], oc[:])
                    # normalize: rows 64:128 of outp hold the denominator.
                    # reciprocal_approx_fast mis-reads when in/out partition
                    # bases differ, so run it over all 128 rows (rows 0:64
                    # are recip of the unnormalized output — unused).
                    for hh in range(2):
                        hp = DH * hh
                        recip = stage.tile([P, SB], F32, tag="recip",
                                           name="recip")
                        nc.vector.reciprocal_approx_fast(recip[:], outp[hh][:])
                        nc.vector.tensor_mul(
                            headsT_sb[hp:hp + DH, hc, j * SB:(j + 1) * SB],
                            outp[hh][0:DH, :], recip[DH:P, :])
                for k in sorted(fillers):
                    for f in fillers[k]:
                        f()

            def F(fn, *a):
                return lambda: fn(*a)

            # ---- schedule: slabs in descending work order; projection and
            # output-projection units woven into attention iterations so the
            # PE stays dense while ACT (exp) is the bottleneck.
            # PE warmup: dummy matmuls run while the first DMAs land and
            # flip the HAM clock gate to 2.4 GHz before real work arrives
            def ka(n, w=P):
                # keep-alive matmuls: no data deps, so they run the moment
                # the PE FIFO reaches them.  A short (N=128) pulse is enough
                # to break the HAM idle window during a DMA-arrival stall;
                # only the initial warm-up burst needs sustained busy (w=SB).
                for i in range(n):
                    wps = pp.tile([P, SB], F32, tag="pp", name="warm_ps")
                    nc.tensor.matmul(wps[:, 0:w], lhsT=wsrc[:, 0:P],
                                     rhs=wsrc[:, 0:w], start=True, stop=True)

            ka(12, w=SB)
            proj_unit("k", 0, 0)
            ka(8, w=SB)
            proj_unit("q", 0, 3)
            # Filler placement rule: each unit is emitted at (or before) the
            # iteration whose instructions first consume its output —
            # scores(hc, tb) need kT/qT of j-slab tb//4, P@V(hc0, tb) needs
            # v tiles tb..tb+1 (iteration index tb//2).
            attn_slab(3, {
                0: [F(ka, 1), F(vproj_unit, 0)],
                1: [F(ka, 1), F(vproj_unit, 2), F(proj_unit, "k", 0, 1),
                    F(proj_unit, "q", 1, 3)],
                2: [F(ka, 1), F(vproj_unit, 4)],
                3: [F(ka, 1), F(vproj_unit, 6), F(proj_unit, "k", 0, 2)],
                4: [F(ka, 1), F(vproj_unit, 8)],
                5: [F(ka, 1), F(vproj_unit, 10), F(proj_unit, "k", 0, 3),
                    F(proj_unit, "k", 1, 0)],
                6: [F(ka, 1), F(vproj_unit, 12)],
                7: [F(ka, 1), F(vproj_unit, 14)],
                9: [F(proj_unit, "k", 1, 1), F(proj_unit, "q", 0, 2)],
                11: [F(proj_unit, "k", 1, 2)],
                13: [F(proj_unit, "k", 1, 3)],
            })
            attn_slab(2, {
                0: [F(oproj_unit, 12)],
                1: [F(oproj_unit, 13)],
                2: [F(oproj_unit, 14), F(proj_unit, "q", 1, 2)],
                3: [F(oproj_unit, 15)],
                6: [F(proj_unit, "q", 0, 1)],
                8: [F(proj_unit, "q", 1, 1)],
            })
            attn_slab(1, {
                0: [F(oproj_unit, 8)], 1: [F(oproj_unit, 9)],
                2: [F(oproj_unit, 10)], 3: [F(oproj_unit, 11)],
                5: [F(proj_unit, "q", 0, 0)],
                6: [F(proj_unit, "q", 1, 0)],
            })
            attn_slab(0, {
                0: [F(ka, 1), F(oproj_unit, 4)],
                1: [F(ka, 1), F(oproj_unit, 5)],
                2: [F(ka, 1), F(oproj_unit, 6)],
                3: [F(ka, 1), F(oproj_unit, 7)],
            })
            for st in range(4):
                ka(1)
                oproj_unit(st, pool=sps)
            ka(2)

            if dump:
                nc.scalar.dma_start(qT_o, qT_sb[:])
                nc.scalar.dma_start(kT_o, kT_sb[:])
                nc.scalar.dma_start(v_o, v_sb[:])
                nc.scalar.dma_start(vbf_o, vbf_sb[:])
                nc.scalar.dma_start(heads_o, headsT_sb[:])
                nc.scalar.dma_start(strips_o, strips_sb[:])

    nc.compile()
    return nc


@functools.lru_cache(maxsize=4)
def _get(mask_mode: str):
    return _build(mask_mode)


def _bf16(a):
    return np.ascontiguousarray(a.astype(ml_dtypes.bfloat16))


def _detect_mask_mode(m):
    if (m == 1).all():
        return "none"
    idx = np.arange(m.shape[0])
    if np.array_equal(m != 0, idx[None, :] <= idx[:, None]):
        return "causal"
    return "generic"


def _strips():
    p = np.arange(P)[:, None]
    f = np.arange(SB)[None, :]
    s = np.stack([(p <= f - P * i) for i in range(SB // P)], axis=1)
    return np.ascontiguousarray(s.astype(ml_dtypes.bfloat16))


def prepare(query, key, value, mask, Wq, bq, Wk, bk, Wv, bv, Wo, bo):
    """Returns (mask_mode, in_maps) for run_bass_kernel_spmd."""
    query = np.asarray(query, dtype=np.float32)
    key = np.asarray(key, dtype=np.float32)
    value = np.asarray(value, dtype=np.float32)
    m2d = np.asarray(mask).reshape(np.asarray(mask).shape[-2:])
    mask_mode = _detect_mask_mode(m2d)

    def prep_x(x):    # [S, D] -> 512-blocked transposed [NSLAB, P, DK*SB] f32
        # xT[b, p, o*SB + c] = x[SB*b + c, P*o + p]
        xt = np.asarray(x, np.float32).T.reshape(DK, P, NSLAB, SB)
        return np.ascontiguousarray(
            xt.transpose(2, 1, 0, 3).reshape(NSLAB, P, DK * SB))

    def _fp8(a):
        return np.ascontiguousarray(a.astype(ml_dtypes.float8_e4m3))

    xq = [_bf16(prep_x(query[b])) for b in range(B)]
    xk = [_bf16(prep_x(key[b])) for b in range(B)]
    xvf = [prep_x(value[b]) for b in range(B)]
    xv0 = [_bf16(xvf[b][0]) for b in range(B)]
    xv8 = [_fp8(xvf[b][1:]) for b in range(B)]

    def prep_w(W, g):     # rows [256g, 256g+256) of W, transposed -> [128, 8, 256]
        sl = np.asarray(W, np.float32)[g * C:(g + 1) * C, :].T
        return _bf16(sl.reshape(DK, P, C).transpose(1, 0, 2))

    def prep_wo(g):       # Wo[:, 256g:256g+256].T -> [128, 2, 1024]
        sl = np.asarray(Wo, np.float32)[:, g * C:(g + 1) * C].T
        return _bf16(sl.reshape(C // P, P, D).transpose(1, 0, 2))

    def prep_b(b_, g):
        sl = np.asarray(b_, np.float32)[g * C:(g + 1) * C]
        return np.ascontiguousarray(sl.reshape(C // P, P).T)

    def prep_bvb(g):
        sl = np.asarray(bv, np.float32)[g * C:(g + 1) * C]
        return np.ascontiguousarray(np.broadcast_to(sl[None, :], (P, C)))

    strips = _strips()
    maskT = _bf16(m2d.T.astype(np.float32)) if mask_mode == "generic" else None

    in_maps = []
    for c in range(NCORES):
        b, g = c // GROUPS, c % GROUPS
        cb = np.concatenate([
            prep_w(Wq, g).reshape(P, 2048), prep_w(Wk, g).reshape(P, 2048),
            prep_w(Wv, g).reshape(P, 2048), prep_wo(g).reshape(P, 2048),
            strips.reshape(P, 2048)], axis=1)
        cf = np.concatenate([
            prep_b(bq, g), prep_b(bk, g), prep_bvb(g)], axis=1)
        def prep_w8(W):
            return (np.asarray(W, np.float32)[g * C:(g + 1) * C, :].T
                    .reshape(DK, P, C).transpose(1, 0, 2).reshape(P, DK * C))

        im = dict(xq=xq[b], xk=xk[b], xv0=xv0[b], xv8=xv8[b],
                  cb=np.ascontiguousarray(cb),
                  cb8=_fp8(prep_w8(Wv)),
                  cf=np.ascontiguousarray(cf.astype(np.float32)))
        if maskT is not None:
            im["maskT"] = maskT
        in_maps.append(im)

    return mask_mode, in_maps


def kernel(query, key, value, mask, Wq, bq, Wk, bk, Wv, bv, Wo, bo):
    mask_mode, in_maps = prepare(query, key, value, mask, Wq, bq, Wk, bk,
                                 Wv, bv, Wo, bo)
    nc = _get(mask_mode)
    res = run_bass_kernel_spmd(nc, in_maps, list(range(NCORES)))
    partials = np.stack([res.results[c]["o"].astype(np.float32)
                         for c in range(NCORES)])
    out = partials.reshape(B, GROUPS, S, D).sum(axis=1)
    out = out + np.asarray(bo, np.float32)[None, None, :]
    return out.astype(np.float32)



# revision 79
# speedup vs baseline: 1.0180x; 1.0180x over previous
"""Multi-head attention (B=2, S=2048, D=1024, H=16, causal) on 8 Trainium2 cores.

Sharding: core c handles batch b = c // 4 and head group g = c % 4 (4 heads,
d_model column slice [256*g, 256*g+256)).  QKV projections are computed per
core against the full sequence of its batch; attention runs per head in a
"scores-transposed" [k, q] layout; the output projection produces a per-core
partial [S, D] (bf16) that the host sums over the 4 head-group cores.

v5 perf structure (PE-bound; ~160us on 8 cores):
- x is transposed AND 512-col-blocked on the HOST to [NSLAB, 128, DK*SB]:
  each block is ONE fully-contiguous [128 x 8KB-row] DMA.  DMA triggers
  cost ~730ns of serial queue time each regardless of size, and the DMA
  engines split HBM bandwidth fairly per QUEUE — so ALL input loads go on
  the sync queue as few big triggers in exact need order (arrival order ==
  need order at full 358 GB/s).
- Scores use d-major PSUM tiles (one [P,2,SB] tile per k-tile of a pair,
  banks = heads) with exp per d over both heads: the two 64-row score MMs
  of a d become READY together, so the Tile scheduler keeps them adjacent
  and they run CONCURRENT in alternating PE row groups (h0/h64) — measured
  ~82% of adjacent score MMs start within 6ns.  (Per-hh tiles stagger
  readiness by the previous iteration's exps and the scheduler regroups MMs
  per row group: no concurrency.  One merged 4-bank tile forces scores(i+1)
  to wait the full exp(i): 1.5us PE stalls + HAM re-throttle.)
- P@V runs in fp8e4 DoubleRow perf mode: expT ([P, 2, SB]) and v
  ([P, 2, HC, P]) are pair-indexed along k-tiles; each k-tile PAIR is one
  fused virtual-K=256 matmul at 2x bf16 throughput.  exp writes fp8 directly
  (bias -3.5: device fp8e4 is IEEE e4m3, max 240 WITH an inf encoding; max
  raw causal score ~68 -> exp arg 68/8-3.5=5.0 -> e^5=148 < 240.  At the
  original bias -3 the max exp was 237 — one fp8 step from inf, and an inf
  exp makes outp/denominator inf -> NaN output rows).  The softmax
  denominator comes from 64 ones-columns appended to V.
- The V projection for s-tiles 4-15 runs fp8 DoubleRow (x and Wv fp8 from
  host, o-chunk pairs fused); tiles 0-3 stay bf16 — they feed the all-bf16
  slab-0 attention (fp8 error concentrates in early rows where softmax
  averages few values).  Q/K projections MUST stay bf16: fp8 k alone costs
  ~2.3e-2 rel err (score noise hits early-row softmax hard) vs the 2e-2
  gate.
- Slabs run in DESCENDING order (3,2,1,0); projection / output-projection
  units are WOVEN into the attention iterations as PE filler.  The pending
  P@V is flushed BEFORE each iteration's fillers (it is always data-ready;
  a filler stuck on a DMA arrival would block it in the PE FIFO).
  Keep-alive matmuls (no data deps) bridge DMA-arrival gaps in the lead-in
  and at iteration starts so the HAM clock gate never sees a full idle
  window (which would re-throttle the PE to 1.2 GHz).
- Final 4 output-projection units draw PSUM from the post-exp-idle sps pool
  (4-deep pipelining with pp) and store per 512-half for an earlier finish.
"""

import functools
import os
import numpy as np
import ml_dtypes

import concourse.bass as bass
import concourse.bacc as bacc
import concourse.tile as tile
import concourse.mybir as mybir
from concourse.bass_utils import run_bass_kernel_spmd

dt = mybir.dt
F32 = dt.float32
BF16 = dt.bfloat16
FP8 = dt.float8e4
AFT = mybir.ActivationFunctionType

B, S, D = 2, 2048, 1024
H, DH = 16, 64
NCORES = 8
GROUPS = NCORES // B            # 4 head-groups
HC = H // GROUPS                # 4 heads per core
C = HC * DH                     # 256 = per-core head-column slice
P = 128
DK = D // P                     # 8 d_in chunks
SB = 512                        # q-slab width
NSLAB = S // SB                 # 4
KT = S // P                     # 16 k tiles
SCALE = 1.0 / float(np.sqrt(DH))


def _build(mask_mode: str):
    """mask_mode: 'causal' | 'none' | 'generic'. Returns compiled Bacc."""
    assert mask_mode in ("causal", "none", "generic")
    causal = mask_mode == "causal"
    nc = bacc.Bacc("TRN2", target_bir_lowering=False, debug=False)

    # host-transposed x, 512-col blocked: xT[b, p, o, c] = x[512*b + c, 128*o + p]
    # -> each 512-block is ONE fully-contiguous [128 x 4096] DMA (one trigger).
    xq_d = nc.dram_tensor("xq", [NSLAB, P, DK * SB], BF16, kind="ExternalInput").ap()
    xk_d = nc.dram_tensor("xk", [NSLAB, P, DK * SB], BF16, kind="ExternalInput").ap()
    # xv block 0 stays bf16 (feeds the all-bf16 slab-0 v); blocks 1-3 are fp8
    # so their vproj runs DoubleRow (v is quantized to fp8 for P@V anyway)
    xv0_d = nc.dram_tensor("xv0", [P, DK * SB], BF16, kind="ExternalInput").ap()
    xv8_d = nc.dram_tensor("xv8", [NSLAB - 1, P, DK * SB], FP8,
                           kind="ExternalInput").ap()
    # all bf16 constants packed into one tensor: wq|wk|wv|wo|strips
    cb_d = nc.dram_tensor("cb", [P, 5 * 2048], BF16, kind="ExternalInput").ap()
    cb8_d = nc.dram_tensor("cb8", [P, 2048], FP8, kind="ExternalInput").ap()
    cf_d = nc.dram_tensor("cf", [P, 260], F32, kind="ExternalInput").ap()
    if mask_mode == "generic":
        maskT_d = nc.dram_tensor("maskT", [S, S], BF16, kind="ExternalInput").ap()
    o_d = nc.dram_tensor("o", [S, D], BF16, kind="ExternalOutput").ap()
    dump = os.environ.get("K_DUMP") == "1"
    if dump:
        qT_o = nc.dram_tensor("qT_o", [P, C // P, S], BF16, kind="ExternalOutput").ap()
        kT_o = nc.dram_tensor("kT_o", [P, C // P, S], BF16, kind="ExternalOutput").ap()
        v_o = nc.dram_tensor("v_o", [P, KT // 2, 2, HC, P], FP8, kind="ExternalOutput").ap()
        vbf_o = nc.dram_tensor("vbf_o", [P, 4, HC, P], BF16, kind="ExternalOutput").ap()
        heads_o = nc.dram_tensor("heads_o", [P, C // P, S], BF16, kind="ExternalOutput").ap()
        strips_o = nc.dram_tensor("strips_o", [P, SB // P, SB], BF16, kind="ExternalOutput").ap()
        expT_o = nc.dram_tensor("expT_o", [P, 2, 16, 2, SB], FP8, kind="ExternalOutput").ap()
        spc_o = nc.dram_tensor("spc_o", [P, 8, 2, SB], F32, kind="ExternalOutput").ap()
        outp_o = nc.dram_tensor("outp_o", [P, 2, 2, SB], F32, kind="ExternalOutput").ap()
        ob_o = nc.dram_tensor("ob_o", [P, D], BF16, kind="ExternalOutput").ap()

    with tile.TileContext(nc) as tc:
        with (
            tc.tile_pool(name="consts", bufs=1) as consts,
            tc.tile_pool(name="xT", bufs=3) as xT_pool,
            tc.tile_pool(name="acts", bufs=1) as acts,
            tc.tile_pool(name="expT", bufs=4) as exp_pool,
            tc.tile_pool(name="stage", bufs=2) as stage,
            tc.tile_pool(name="pp", bufs=2, space="PSUM") as pp,
            tc.tile_pool(name="sps", bufs=2, space="PSUM") as sps,
            tc.tile_pool(name="otp", bufs=2, space="PSUM") as otp,
        ):
            # ---- constants: one dedicated tile per constant, each written by
            # exactly one whole-tile DMA (readers then depend on the whole
            # write — no sub-range matching through view slices).
            wk_t = consts.tile([P, DK, C], BF16, name="wk")
            wq_t = consts.tile([P, DK, C], BF16, name="wq")
            wv_t = consts.tile([P, DK, C], BF16, name="wv")
            wv8_t = consts.tile([P, DK, C], FP8, name="wv8")
            wo_t = consts.tile([P, C // P, D], BF16, name="wo")
            strips_t = consts.tile([P, SB // P, SB], BF16, name="strips")
            cf_sb = consts.tile([P, 260], F32)
            # weights go FIRST on the sync HWDGE ring (SWDGE is slow to
            # start and wk gates the very first projection)
            wq_sb, wk_sb, wv_sb = wq_t[:], wk_t[:], wv_t[:]
            wv8_sb = wv8_t[:]
            wo_sb, strips_sb = wo_t[:], strips_t[:]
            bq_sb = cf_sb[:, 0:2]
            bk_sb = cf_sb[:, 2:4]
            bvb_sb = cf_sb[:, 4:260]
            # warmup source: the very first DVE op so the PE warmup
            # matmuls can start as soon as the engines come up
            wsrc = stage.tile([P, SB], BF16, tag="warm", name="wsrc")
            nc.vector.memset(wsrc[:], 0.5)
            # bias -3.5: max exp e^5.0 = 148 < fp8e4 max 240 (see module
            # docstring — bias -3 sat one fp8 step below the inf encoding).
            # NOT lower: typical exp values must stay in fp8 NORMAL range
            # (>= 2^-6); bias -4 pushed them subnormal (rel err 6e-3 -> 2e-2).
            expbias_sb = consts.tile([P, 1], F32)
            nc.vector.memset(expbias_sb[:], -3.5)

            # ---- x loads.  Each DMA trigger costs ~730ns of serial queue
            # time regardless of size, so x moves as ONE contiguous trigger
            # per 512-col block (the consumption granule), ALL on the sync
            # queue in exact need order (see module docstring: per-queue
            # fair-share bandwidth makes a second queue counterproductive).
            xqT = xT_pool.tile([P, NSLAB, DK, SB], BF16, tag="xT", name="xqT")
            xkT = xT_pool.tile([P, NSLAB, DK, SB], BF16, tag="xT", name="xkT")
            xv0T = xT_pool.tile([P, DK, SB], BF16, tag="xT", name="xv0T")
            xv8T = xT_pool.tile([P, NSLAB - 1, DK, SB], FP8, tag="xT",
                                name="xv8T")

            def ld(eng, xt, x_d, b):
                eng.dma_start(xt[:, b], x_d[b])

            # ONE queue, exact need order: the DMA engines split HBM
            # bandwidth fairly per QUEUE, so a second queue running early
            # non-critical transfers steals bandwidth from the critical
            # stream.  Within a queue, transfers complete in order at full
            # rate — need-order IS arrival-order.
            nc.sync.dma_start(wk_t[:], cb_d[:, 2048:4096])
            ld(nc.sync, xkT, xk_d, 0)
            nc.sync.dma_start(wq_t[:], cb_d[:, 0:2048])
            nc.sync.dma_start(cf_sb[:], cf_d)
            ld(nc.sync, xqT, xq_d, 3)
            nc.sync.dma_start(wv_t[:], cb_d[:, 4096:6144])
            nc.sync.dma_start(xv0T[:], xv0_d)
            nc.sync.dma_start(wv8_t[:], cb8_d)
            ld(nc.sync, xkT, xk_d, 1)
            nc.sync.dma_start(xv8T[:, 0], xv8_d[0])
            ld(nc.sync, xkT, xk_d, 2)
            nc.sync.dma_start(xv8T[:, 1], xv8_d[1])
            ld(nc.sync, xkT, xk_d, 3)
            nc.sync.dma_start(xv8T[:, 2], xv8_d[2])
            nc.sync.dma_start(wo_t[:], cb_d[:, 6144:8192])
            ld(nc.sync, xqT, xq_d, 2)
            ld(nc.sync, xqT, xq_d, 1)
            ld(nc.sync, xqT, xq_d, 0)

            qT_sb = acts.tile([P, C // P, S], BF16)       # [d_out, s] head-major
            kT_sb = acts.tile([P, C // P, S], BF16)
            headsT_sb = acts.tile([P, C // P, S], BF16)
            # v pair-indexed fp8: [p, kpair, h, i, col]; cols 0:64 v, 64:128
            # ones.  The h-major layout makes the DoubleRow lhsT slice
            # [:, kp, h, :, :] a contiguous trailing block (dependency
            # tracking of the interleaved [:, kp, :, h, :] form missed the
            # vproj writes -> first-execution read-before-write NaN).
            v_sb = acts.tile([P, KT // 2, HC, 2, P], FP8)
            nc.gpsimd.memset(v_sb[:, :, :, :, DH:P], 1.0)
            # bf16 copy of v k-tiles 0..3 for the all-bf16 slab-0 attention
            vbf_sb = acts.tile([P, 4, HC, P], BF16)
            nc.gpsimd.memset(vbf_sb[:, :, :, DH:P], 1.0)
            # causal strips generated on-device (saves 0.5MB of critical
            # startup DMA): strips[p, i, f] = 1.0 where p <= f - 128*i.
            # iota = f - 128*i - p; bf16 iota is inexact above |256| but the
            # comparison boundary (|iota| small) is exact.  gpsimd is the
            # only engine with affine_select; it is idle this early and
            # strips aren't consumed until slab 3's first diagonal pair.
            nc.gpsimd.memset(strips_t[:], 1.0)
            nc.gpsimd.affine_select(
                strips_t[:], strips_t[:],
                pattern=[[-P, SB // P], [1, SB]],
                compare_op=mybir.AluOpType.is_ge, fill=0.0,
                base=0, channel_multiplier=-1)

            def proj_unit(which, co, j):
                ps = pp.tile([P, SB], F32, tag="pp", name="proj_ps")
                if which == "q":
                    b_sb, outT = bq_sb, qT_sb
                    for o in range(DK):
                        nc.tensor.matmul(
                            ps[:],
                            lhsT=wq_sb[:, o, co * P:(co + 1) * P],
                            rhs=xqT[:, j, o, :],
                            start=(o == 0), stop=(o == DK - 1))
                else:
                    b_sb, outT = bk_sb, kT_sb
                    for o in range(DK):
                        nc.tensor.matmul(
                            ps[:],
                            lhsT=wk_sb[:, o, co * P:(co + 1) * P],
                            rhs=xkT[:, j, o, :],
                            start=(o == 0), stop=(o == DK - 1))
                nc.vector.tensor_scalar_add(
                    outT[:, co, j * SB:(j + 1) * SB], ps[:], b_sb[:, co:co + 1])

            def vproj_unit(st0, nst=2):
                for st in range(st0, st0 + nst):
                    ps = pp.tile([P, SB], F32, tag="pp", name="vproj_ps")
                    c0, c1 = (st % 4) * P, (st % 4 + 1) * P
                    if st < 4:
                        for o in range(DK):
                            nc.tensor.matmul(
                                ps[:, 0:C],
                                lhsT=xv0T[:, o, c0:c1],
                                rhs=wv_sb[:, o, :],
                                start=(o == 0), stop=(o == DK - 1))
                    else:
                        # fp8 x + fp8 w, o-chunk pairs fused via DoubleRow
                        for op in range(DK // 2):
                            nc.tensor.matmul(
                                ps[:, 0:C],
                                lhsT=xv8T[:, st // 4 - 1,
                                          2 * op:2 * op + 2, c0:c1],
                                rhs=wv8_sb[:, 2 * op:2 * op + 2, :],
                                start=(op == 0), stop=(op == DK // 2 - 1),
                                perf_mode=mybir.MatmulPerfMode.DoubleRow)
                    # one contiguous DVE add into a staging tile, then per-
                    # head gpsimd copies into v.  Per-head contiguous writes
                    # are required: a single strided (h-step 256) write is
                    # NOT seen as overlapping the DoubleRow lhsT read by the
                    # dependency tracker (first-exec read-before-write NaN);
                    # gpsimd does them so the DVE isn't op-count-bound.
                    vstage = stage.tile([P, C], BF16, tag="vst", name="vst")
                    nc.vector.tensor_add(vstage[:], ps[:, 0:C], bvb_sb[:])
                    for h in range(HC):
                        nc.gpsimd.tensor_copy(
                            v_sb[:, st // 2, h, st % 2, 0:DH],
                            vstage[:, h * DH:(h + 1) * DH])
                        if st < 4:
                            nc.gpsimd.tensor_copy(
                                vbf_sb[:, st, h, 0:DH],
                                vstage[:, h * DH:(h + 1) * DH])

            def oproj_unit(st, pool=None):
                ob = stage.tile([P, D], BF16, tag="ob", name="ob")
                if pool is None:
                    ps = {n2: pp.tile([P, SB], F32, tag="pp", name="o_ps")
                          for n2 in range(D // SB)}
                else:
                    # final units draw PSUM from the sps pool (free after the
                    # last exp): with pp's 2 bufs ALSO free this pipelines
                    # deeper instead of stalling on each unit's drains
                    pst = pool.tile([P, 2, SB], F32, tag="sps", name="o_ps2")
                    ps = {n2: pst[:, n2, :] for n2 in range(D // SB)}
                # cc-outer: both n2 matmuls of a cc share the stationary
                for cc in range(C // P):
                    for n2 in range(D // SB):
                        nc.tensor.matmul(
                            ps[n2][:],
                            lhsT=headsT_sb[:, cc, st * P:(st + 1) * P],
                            rhs=wo_sb[:, cc, n2 * SB:(n2 + 1) * SB],
                            start=(cc == 0), stop=(cc == C // P - 1))
                for n2 in range(D // SB):
                    # tail slabs (st<8 run during/after slab0): ACT is idle
                    # after the last exp while DVE still has normalize work —
                    # split the two PSUM->SBUF drains across both engines.
                    if st < 8 and n2 == 0:
                        nc.scalar.copy(ob[:, n2 * SB:(n2 + 1) * SB], ps[n2][:])
                    else:
                        nc.vector.tensor_copy(ob[:, n2 * SB:(n2 + 1) * SB],
                                              ps[n2][:])
                # stores go on the sync queue (idle once the input loads are
                # issued); a store trigger on the scalar queue would block
                # subsequent exp instructions — strict FIFO.  The final units
                # (sps pool) store per half so the first half streams out
                # while the second is still copying.
                if pool is None:
                    nc.sync.dma_start(o_d[st * P:(st + 1) * P, :], ob[:])
                else:
                    for n2 in range(D // SB):
                        nc.sync.dma_start(
                            o_d[st * P:(st + 1) * P, n2 * SB:(n2 + 1) * SB],
                            ob[:, n2 * SB:(n2 + 1) * SB])
                if dump and st == 12:
                    nc.sync.dma_start(ob_o, ob[:])

            def attn_slab(j, fillers=None):
                # fillers: {pair_index: [closures]}.  Tile derives
                # dependencies from EMISSION order (a reader only waits on
                # earlier-emitted writers), so every filler must be emitted
                # before its first consumer — never after it.
                fillers = dict(fillers or {})
                n_kt = 4 * (j + 1) if causal else KT
                bfslab = causal and j == 0    # slab 0 all-bf16
                edt = BF16 if bfslab else FP8
                etag = "expTbf" if bfslab else "expT"
                it = 0
                for hc in range(HC // 2):          # head pair (2hc, 2hc+1)
                    outp = [otp.tile([P, SB], F32, tag="otp", name=f"outp{hh}")
                            for hh in range(2)]
                    pend = [None, None]

                    def qlo(t):   # valid-q offset within slab for k-tile t
                        if not causal:
                            return 0
                        return max(0, P * t - SB * j)

                    def make_pav(tb, qp, hh, expT_t):
                        # expT_t is the iteration's [P, 2(d), 2(hh), SB]
                        # tile; head hh's k-tile pair is the strided slice
                        # [:, :, hh, :]
                        h = 2 * hc + hh

                        def pav():
                            if bfslab:
                                for d_ in range(2):
                                    t = tb + d_
                                    ql = qlo(t)
                                    nc.tensor.matmul(
                                        outp[hh][:, ql:],
                                        lhsT=vbf_sb[:, t, h, :],
                                        rhs=expT_t[:, d_, hh, ql:],
                                        start=(t == 0), stop=(t == n_kt - 1))
                            elif (causal and tb >= 4 * j) or \
                                    mask_mode == "generic":
                                # masked pairs run as 2 plain fp8 matmuls:
                                # their [d_, hh, qp:] rhs reads structurally
                                # match the strip/mask tensor_mul writes, so
                                # the dependency tracker sees them
                                for d_ in range(2):
                                    nc.tensor.matmul(
                                        outp[hh][:, qp:],
                                        lhsT=v_sb[:, tb // 2, h, d_, :],
                                        rhs=expT_t[:, d_, hh, qp:],
                                        start=(tb == 0 and d_ == 0),
                                        stop=(d_ == 1 and tb == n_kt - 2))
                            else:
                                # fused P@V: one DoubleRow matmul per k-tile
                                # pair (2 fp8 weights/cell; virtual K=256)
                                nc.tensor.matmul(
                                    outp[hh][:, qp:],
                                    lhsT=v_sb[:, tb // 2, h, :, :],
                                    rhs=expT_t[:, :, hh, qp:],
                                    start=(tb == 0), stop=(tb == n_kt - 2),
                                    perf_mode=mybir.MatmulPerfMode.DoubleRow)
                        return pav

                    # d-major iteration: one [P,2,SB] PSUM tile per k-tile of
                    # the pair, each holding BOTH heads' scores (bank=hh).
                    # The 2 MMs of a d become READY together (single WAR edge
                    # on that d's exp), so the scheduler keeps them adjacent
                    # and — being different row groups (h0/h64) — they run
                    # CONCURRENT in the array.  (With per-hh tiles, readiness
                    # was staggered by the previous iteration's exps and the
                    # scheduler regrouped the MMs per row group — no
                    # concurrency.  With one merged 4-bank tile, scores(i+1)
                    # had to wait the FULL exp(i) — 1.5us PE stalls and HAM
                    # re-throttle.)  exp runs per d over both heads (N=1024);
                    # expT is [P, 2(d), 2(hh), SB] so the DR P@V rhs is the
                    # clean strided slice [:, :, hh, qp:].
                    for tb in range(0, n_kt, 2):
                        myfill = fillers.pop(it, ())
                        it += 1
                        qb = qlo(tb)
                        qp = qb
                        mloads = []
                        # both tiles write from the PAIR base qb: exp reads
                        # [qb:], and a never-written stale PSUM value there
                        # can exp() to inf -> fp8 INF -> NaN via the
                        # strip-zero multiply.
                        spd = [sps.tile([P, 2, SB], F32, tag="sps", name="sp")
                               for _ in range(2)]
                        expT4 = exp_pool.tile([P, 2, 2, SB], edt, tag=etag,
                                              name="expT")
                        for d_ in range(2):
                            t = tb + d_
                            for hh in range(2):
                                hp = DH * hh
                                nc.tensor.matmul(
                                    spd[d_][:, hh, qb:],
                                    lhsT=kT_sb[hp:hp + DH, hc,
                                               t * P:(t + 1) * P],
                                    rhs=qT_sb[hp:hp + DH, hc,
                                              j * SB + qb:(j + 1) * SB],
                                    start=True, stop=True,
                                    tile_position=(hp, 0))
                        # bias -3.5: keeps exp within fp8e4 range (softmax
                        # is shift-invariant; numerator and denominator
                        # share the e^-3.5 factor)
                        for d_ in range(2):
                            nc.scalar.activation(
                                expT4[:, d_, :, qb:],
                                spd[d_][:, 0:2, qb:], AFT.Exp, scale=SCALE,
                                bias=expbias_sb[:])
                        if dump and j == 3 and hc == 0 and not bfslab:
                            for hh in range(2):
                                nc.sync.dma_start(
                                    expT_o[:, hh, tb // 2, :, :],
                                    expT4[:, :, hh, :])
                        # the pending P@V (previous pair's) goes BEFORE the
                        # fillers: it is always data-ready (its v tiles came
                        # from the previous iteration), while a filler stuck
                        # on a DMA arrival would block it in the PE FIFO and
                        # idle the array long enough to re-throttle the clock
                        for hh in range(2):
                            if pend[hh] is not None:
                                pend[hh]()
                                pend[hh] = None
                        # fillers after this pair's scores (so they are not
                        # queued ahead of them) but before the P@V that may
                        # consume their output
                        for f in myfill:
                            f()
                        for d_ in range(2):
                            t = tb + d_
                            if mask_mode == "generic":
                                m_sb = stage.tile([P, SB], BF16, tag="msk",
                                                  name="m_sb")
                                nc.gpsimd.dma_start(
                                    m_sb[:],
                                    maskT_d[t * P:(t + 1) * P,
                                            j * SB:(j + 1) * SB])
                                mloads.append(m_sb)
                            if mask_mode == "generic":
                                for hh in range(2):
                                    nc.vector.tensor_mul(
                                        expT4[:, d_, hh, :],
                                        expT4[:, d_, hh, :],
                                        mloads[d_][:])
                            elif causal and t >= 4 * j:
                                # mask relative to the PAIR's q window:
                                # d_=0: triangle on 128 cols; d_=1: zero
                                # prefix + triangle over 256 cols; one mul
                                # covers BOTH heads' banks of this d
                                w = min((d_ + 1) * P, SB - qp)
                                for hh in range(2):
                                    nc.vector.tensor_mul(
                                        expT4[:, d_, hh, qp:qp + w],
                                        expT4[:, d_, hh, qp:qp + w],
                                        strips_sb[:, d_, 0:w])
                        for hh in range(2):
                            if pend[hh] is not None:
                                pend[hh]()
                            pend[hh] = make_pav(tb, qp, hh, expT4)
                    for hh in range(2):
                        pend[hh]()
                    if dump and j == 3 and hc == 0:
                        for hh in range(2):
                            oc = stage.tile([P, SB], F32, tag="recip",
                                            name="oc_dump")
                            nc.vector.tensor_copy(oc[:], outp[hh][:])
                            nc.sync.dma_start(outp_o[:, hh, 0, :], oc[:])
                    # normalize: rows 64:128 of outp hold the denominator.
                    # reciprocal_approx_fast mis-reads when in/out partition
                    # bases differ, so run it over all 128 rows (rows 0:64
                    # are recip of the unnormalized output — unused).
                    for hh in range(2):
                        hp = DH * hh
                        recip = stage.tile([P, SB], F32, tag="recip",
                                           name="recip")
                        nc.vector.reciprocal_approx_fast(recip[:], outp[hh][:])
                        nc.vector.tensor_mul(
                            headsT_sb[hp:hp + DH, hc, j * SB:(j + 1) * SB],
                            outp[hh][0:DH, :], recip[DH:P, :])
                for k in sorted(fillers):
                    for f in fillers[k]:
                        f()

            def F(fn, *a):
                return lambda: fn(*a)

            # ---- schedule: slabs in descending work order; projection and
            # output-projection units woven into attention iterations so the
            # PE stays dense while ACT (exp) is the bottleneck.
            # PE warmup: dummy matmuls run while the first DMAs land and
            # flip the HAM clock gate to 2.4 GHz before real work arrives
            def ka(n, w=P):
                # keep-alive matmuls: no data deps, so they run the moment
                # the PE FIFO reaches them.  A short (N=128) pulse is enough
                # to break the HAM idle window during a DMA-arrival stall;
                # only the initial warm-up burst needs sustained busy (w=SB).
                for i in range(n):
                    wps = pp.tile([P, SB], F32, tag="pp", name="warm_ps")
                    nc.tensor.matmul(wps[:, 0:w], lhsT=wsrc[:, 0:P],
                                     rhs=wsrc[:, 0:w], start=True, stop=True)

            ka(12, w=SB)
            proj_unit("k", 0, 0)
            ka(8, w=SB)
            proj_unit("q", 0, 3)
            # Filler placement rule: each unit is emitted at (or before) the
            # iteration whose instructions first consume its output —
            # scores(hc, tb) need kT/qT of j-slab tb//4, P@V(hc0, tb) needs
            # v tiles tb..tb+1 (iteration index tb//2).
            attn_slab(3, {
                0: [F(ka, 1), F(vproj_unit, 0)],
                1: [F(ka, 1), F(vproj_unit, 2), F(proj_unit, "k", 0, 1),
                    F(proj_unit, "q", 1, 3)],
                2: [F(ka, 1), F(vproj_unit, 4)],
                3: [F(ka, 1), F(vproj_unit, 6), F(proj_unit, "k", 0, 2)],
                4: [F(ka, 1), F(vproj_unit, 8)],
                5: [F(ka, 1), F(vproj_unit, 10), F(proj_unit, "k", 0, 3),
                    F(proj_unit, "k", 1, 0)],
                6: [F(ka, 1), F(vproj_unit, 12)],
                7: [F(ka, 1), F(vproj_unit, 14)],
                9: [F(proj_unit, "k", 1, 1), F(proj_unit, "q", 0, 2)],
                11: [F(proj_unit, "k", 1, 2)],
                13: [F(proj_unit, "k", 1, 3)],
            })
            attn_slab(2, {
                0: [F(oproj_unit, 12)],
                1: [F(oproj_unit, 13)],
                2: [F(oproj_unit, 14), F(proj_unit, "q", 1, 2)],
                3: [F(oproj_unit, 15)],
                6: [F(proj_unit, "q", 0, 1)],
                8: [F(proj_unit, "q", 1, 1)],
            })
            attn_slab(1, {
                0: [F(oproj_unit, 8)], 1: [F(oproj_unit, 9)],
                2: [F(oproj_unit, 10)], 3: [F(oproj_unit, 11)],
                5: [F(proj_unit, "q", 0, 0)],
                6: [F(proj_unit, "q", 1, 0)],
            })
            attn_slab(0, {
                0: [F(ka, 1), F(oproj_unit, 4)],
                1: [F(ka, 1), F(oproj_unit, 5)],
                2: [F(ka, 1), F(oproj_unit, 6)],
                3: [F(ka, 1), F(oproj_unit, 7)],
            })
            for st in range(4):
                ka(1)
                oproj_unit(st, pool=sps)
            ka(2)

            if dump:
                nc.scalar.dma_start(qT_o, qT_sb[:])
                nc.scalar.dma_start(kT_o, kT_sb[:])
                nc.scalar.dma_start(v_o, v_sb[:])
                nc.scalar.dma_start(vbf_o, vbf_sb[:])
                nc.scalar.dma_start(heads_o, headsT_sb[:])
                nc.scalar.dma_start(strips_o, strips_sb[:])

    nc.compile()
    return nc


@functools.lru_cache(maxsize=4)
def _get(mask_mode: str):
    return _build(mask_mode)


def _bf16(a):
    return np.ascontiguousarray(a.astype(ml_dtypes.bfloat16))


def _detect_mask_mode(m):
    if (m == 1).all():
        return "none"
    idx = np.arange(m.shape[0])
    if np.array_equal(m != 0, idx[None, :] <= idx[:, None]):
        return "causal"
    return "generic"


def _strips():
    p = np.arange(P)[:, None]
    f = np.arange(SB)[None, :]
    s = np.stack([(p <= f - P * i) for i in range(SB // P)], axis=1)
    return np.ascontiguousarray(s.astype(ml_dtypes.bfloat16))


def prepare(query, key, value, mask, Wq, bq, Wk, bk, Wv, bv, Wo, bo):
    """Returns (mask_mode, in_maps) for run_bass_kernel_spmd."""
    query = np.asarray(query, dtype=np.float32)
    key = np.asarray(key, dtype=np.float32)
    value = np.asarray(value, dtype=np.float32)
    m2d = np.asarray(mask).reshape(np.asarray(mask).shape[-2:])
    mask_mode = _detect_mask_mode(m2d)

    def prep_x(x):    # [S, D] -> 512-blocked transposed [NSLAB, P, DK*SB] f32
        # xT[b, p, o*SB + c] = x[SB*b + c, P*o + p]
        xt = np.asarray(x, np.float32).T.reshape(DK, P, NSLAB, SB)
        return np.ascontiguousarray(
            xt.transpose(2, 1, 0, 3).reshape(NSLAB, P, DK * SB))

    def _fp8(a):
        return np.ascontiguousarray(a.astype(ml_dtypes.float8_e4m3))

    xq = [_bf16(prep_x(query[b])) for b in range(B)]
    xk = [_bf16(prep_x(key[b])) for b in range(B)]
    xvf = [prep_x(value[b]) for b in range(B)]
    xv0 = [_bf16(xvf[b][0]) for b in range(B)]
    xv8 = [_fp8(xvf[b][1:]) for b in range(B)]

    def prep_w(W, g):     # rows [256g, 256g+256) of W, transposed -> [128, 8, 256]
        sl = np.asarray(W, np.float32)[g * C:(g + 1) * C, :].T
        return _bf16(sl.reshape(DK, P, C).transpose(1, 0, 2))

    def prep_wo(g):       # Wo[:, 256g:256g+256].T -> [128, 2, 1024]
        sl = np.asarray(Wo, np.float32)[:, g * C:(g + 1) * C].T
        return _bf16(sl.reshape(C // P, P, D).transpose(1, 0, 2))

    def prep_b(b_, g):
        sl = np.asarray(b_, np.float32)[g * C:(g + 1) * C]
        return np.ascontiguousarray(sl.reshape(C // P, P).T)

    def prep_bvb(g):
        sl = np.asarray(bv, np.float32)[g * C:(g + 1) * C]
        return np.ascontiguousarray(np.broadcast_to(sl[None, :], (P, C)))

    strips = _strips()
    maskT = _bf16(m2d.T.astype(np.float32)) if mask_mode == "generic" else None

    in_maps = []
    for c in range(NCORES):
        b, g = c // GROUPS, c % GROUPS
        cb = np.concatenate([
            prep_w(Wq, g).reshape(P, 2048), prep_w(Wk, g).reshape(P, 2048),
            prep_w(Wv, g).reshape(P, 2048), prep_wo(g).reshape(P, 2048),
            strips.reshape(P, 2048)], axis=1)
        cf = np.concatenate([
            prep_b(bq, g), prep_b(bk, g), prep_bvb(g)], axis=1)
        def prep_w8(W):
            return (np.asarray(W, np.float32)[g * C:(g + 1) * C, :].T
                    .reshape(DK, P, C).transpose(1, 0, 2).reshape(P, DK * C))

        im = dict(xq=xq[b], xk=xk[b], xv0=xv0[b], xv8=xv8[b],
                  cb=np.ascontiguousarray(cb),
                  cb8=_fp8(prep_w8(Wv)),
                  cf=np.ascontiguousarray(cf.astype(np.float32)))
        if maskT is not None:
            im["maskT"] = maskT
        in_maps.append(im)

    return mask_mode, in_maps


def kernel(query, key, value, mask, Wq, bq, Wk, bk, Wv, bv, Wo, bo):
    mask_mode, in_maps = prepare(query, key, value, mask, Wq, bq, Wk, bk,
                                 Wv, bv, Wo, bo)
    nc = _get(mask_mode)
    res = run_bass_kernel_spmd(nc, in_maps, list(range(NCORES)))
    partials = np.stack([res.results[c]["o"].astype(np.float32)
                         for c in range(NCORES)])
    out = partials.reshape(B, GROUPS, S, D).sum(axis=1)
    out = out + np.asarray(bo, np.float32)[None, None, :]
    return out.astype(np.float32)



# revision 82
# speedup vs baseline: 1.0185x; 1.0005x over previous
"""Multi-head attention (B=2, S=2048, D=1024, H=16, causal) on 8 Trainium2 cores.

Sharding: core c handles batch b = c // 4 and head group g = c % 4 (4 heads,
d_model column slice [256*g, 256*g+256)).  QKV projections are computed per
core against the full sequence of its batch; attention runs per head in a
"scores-transposed" [k, q] layout; the output projection produces a per-core
partial [S, D] (bf16) that the host sums over the 4 head-group cores.

v5 perf structure (PE-bound; ~160us on 8 cores):
- x is transposed AND 512-col-blocked on the HOST to [NSLAB, 128, DK*SB]:
  each block is ONE fully-contiguous [128 x 8KB-row] DMA.  DMA triggers
  cost ~730ns of serial queue time each regardless of size, and the DMA
  engines split HBM bandwidth fairly per QUEUE — so ALL input loads go on
  the sync queue as few big triggers in exact need order (arrival order ==
  need order at full 358 GB/s).
- Scores use d-major PSUM tiles (one [P,2,SB] tile per k-tile of a pair,
  banks = heads) with exp per d over both heads: the two 64-row score MMs
  of a d become READY together, so the Tile scheduler keeps them adjacent
  and they run CONCURRENT in alternating PE row groups (h0/h64) — measured
  ~82% of adjacent score MMs start within 6ns.  (Per-hh tiles stagger
  readiness by the previous iteration's exps and the scheduler regroups MMs
  per row group: no concurrency.  One merged 4-bank tile forces scores(i+1)
  to wait the full exp(i): 1.5us PE stalls + HAM re-throttle.)
- P@V runs in fp8e4 DoubleRow perf mode: expT ([P, 2, SB]) and v
  ([P, 2, HC, P]) are pair-indexed along k-tiles; each k-tile PAIR is one
  fused virtual-K=256 matmul at 2x bf16 throughput.  exp writes fp8 directly
  (bias -3.5: device fp8e4 is IEEE e4m3, max 240 WITH an inf encoding; max
  raw causal score ~68 -> exp arg 68/8-3.5=5.0 -> e^5=148 < 240.  At the
  original bias -3 the max exp was 237 — one fp8 step from inf, and an inf
  exp makes outp/denominator inf -> NaN output rows).  The softmax
  denominator comes from 64 ones-columns appended to V.
- The V projection for s-tiles 4-15 runs fp8 DoubleRow (x and Wv fp8 from
  host, o-chunk pairs fused); tiles 0-3 stay bf16 — they feed the all-bf16
  slab-0 attention (fp8 error concentrates in early rows where softmax
  averages few values).  Q/K projections MUST stay bf16: fp8 k alone costs
  ~2.3e-2 rel err (score noise hits early-row softmax hard) vs the 2e-2
  gate.
- Slabs run in DESCENDING order (3,2,1,0); projection / output-projection
  units are WOVEN into the attention iterations as PE filler.  The pending
  P@V is flushed BEFORE each iteration's fillers (it is always data-ready;
  a filler stuck on a DMA arrival would block it in the PE FIFO).
  Keep-alive matmuls (no data deps) bridge DMA-arrival gaps in the lead-in
  and at iteration starts so the HAM clock gate never sees a full idle
  window (which would re-throttle the PE to 1.2 GHz).
- Final 4 output-projection units draw PSUM from the post-exp-idle sps pool
  (4-deep pipelining with pp) and store per 512-half for an earlier finish.
"""

import functools
import os
import numpy as np
import ml_dtypes

import concourse.bass as bass
import concourse.bacc as bacc
import concourse.tile as tile
import concourse.mybir as mybir
from concourse.bass_utils import run_bass_kernel_spmd

dt = mybir.dt
F32 = dt.float32
BF16 = dt.bfloat16
FP8 = dt.float8e4
AFT = mybir.ActivationFunctionType

B, S, D = 2, 2048, 1024
H, DH = 16, 64
NCORES = 8
GROUPS = NCORES // B            # 4 head-groups
HC = H // GROUPS                # 4 heads per core
C = HC * DH                     # 256 = per-core head-column slice
P = 128
DK = D // P                     # 8 d_in chunks
SB = 512                        # q-slab width
NSLAB = S // SB                 # 4
KT = S // P                     # 16 k tiles
SCALE = 1.0 / float(np.sqrt(DH))


def _build(mask_mode: str):
    """mask_mode: 'causal' | 'none' | 'generic'. Returns compiled Bacc."""
    assert mask_mode in ("causal", "none", "generic")
    causal = mask_mode == "causal"
    nc = bacc.Bacc("TRN2", target_bir_lowering=False, debug=False)

    # host-transposed x, 512-col blocked: xT[b, p, o, c] = x[512*b + c, 128*o + p]
    # -> each 512-block is ONE fully-contiguous [128 x 4096] DMA (one trigger).
    xq_d = nc.dram_tensor("xq", [NSLAB, P, DK * SB], BF16, kind="ExternalInput").ap()
    xk_d = nc.dram_tensor("xk", [NSLAB, P, DK * SB], BF16, kind="ExternalInput").ap()
    # xv block 0 stays bf16 (feeds the all-bf16 slab-0 v); blocks 1-3 are fp8
    # so their vproj runs DoubleRow (v is quantized to fp8 for P@V anyway)
    xv0_d = nc.dram_tensor("xv0", [P, DK * SB], BF16, kind="ExternalInput").ap()
    xv8_d = nc.dram_tensor("xv8", [NSLAB - 1, P, DK * SB], FP8,
                           kind="ExternalInput").ap()
    # all bf16 constants packed into one tensor: wq|wk|wv|wo|strips
    cb_d = nc.dram_tensor("cb", [P, 5 * 2048], BF16, kind="ExternalInput").ap()
    cb8_d = nc.dram_tensor("cb8", [P, 2048], FP8, kind="ExternalInput").ap()
    cf_d = nc.dram_tensor("cf", [P, 260], F32, kind="ExternalInput").ap()
    if mask_mode == "generic":
        maskT_d = nc.dram_tensor("maskT", [S, S], BF16, kind="ExternalInput").ap()
    o_d = nc.dram_tensor("o", [S, D], BF16, kind="ExternalOutput").ap()
    dump = os.environ.get("K_DUMP") == "1"
    if dump:
        qT_o = nc.dram_tensor("qT_o", [P, C // P, S], BF16, kind="ExternalOutput").ap()
        kT_o = nc.dram_tensor("kT_o", [P, C // P, S], BF16, kind="ExternalOutput").ap()
        v_o = nc.dram_tensor("v_o", [P, KT // 2, 2, HC, P], FP8, kind="ExternalOutput").ap()
        vbf_o = nc.dram_tensor("vbf_o", [P, 4, HC, P], BF16, kind="ExternalOutput").ap()
        heads_o = nc.dram_tensor("heads_o", [P, C // P, S], BF16, kind="ExternalOutput").ap()
        strips_o = nc.dram_tensor("strips_o", [P, SB // P, SB], BF16, kind="ExternalOutput").ap()
        expT_o = nc.dram_tensor("expT_o", [P, 2, 16, 2, SB], FP8, kind="ExternalOutput").ap()
        spc_o = nc.dram_tensor("spc_o", [P, 8, 2, SB], F32, kind="ExternalOutput").ap()
        outp_o = nc.dram_tensor("outp_o", [P, 2, 2, SB], F32, kind="ExternalOutput").ap()
        ob_o = nc.dram_tensor("ob_o", [P, D], BF16, kind="ExternalOutput").ap()

    with tile.TileContext(nc) as tc:
        with (
            tc.tile_pool(name="consts", bufs=1) as consts,
            tc.tile_pool(name="xT", bufs=3) as xT_pool,
            tc.tile_pool(name="acts", bufs=1) as acts,
            tc.tile_pool(name="expT", bufs=4) as exp_pool,
            tc.tile_pool(name="stage", bufs=2) as stage,
            tc.tile_pool(name="pp", bufs=2, space="PSUM") as pp,
            tc.tile_pool(name="sps", bufs=2, space="PSUM") as sps,
            tc.tile_pool(name="otp", bufs=2, space="PSUM") as otp,
        ):
            # ---- constants: one dedicated tile per constant, each written by
            # exactly one whole-tile DMA (readers then depend on the whole
            # write — no sub-range matching through view slices).
            wk_t = consts.tile([P, DK, C], BF16, name="wk")
            wq_t = consts.tile([P, DK, C], BF16, name="wq")
            wv_t = consts.tile([P, DK, C], BF16, name="wv")
            wv8_t = consts.tile([P, DK, C], FP8, name="wv8")
            wo_t = consts.tile([P, C // P, D], BF16, name="wo")
            strips_t = consts.tile([P, SB // P, SB], BF16, name="strips")
            cf_sb = consts.tile([P, 260], F32)
            # weights go FIRST on the sync HWDGE ring (SWDGE is slow to
            # start and wk gates the very first projection)
            wq_sb, wk_sb, wv_sb = wq_t[:], wk_t[:], wv_t[:]
            wv8_sb = wv8_t[:]
            wo_sb, strips_sb = wo_t[:], strips_t[:]
            bq_sb = cf_sb[:, 0:2]
            bk_sb = cf_sb[:, 2:4]
            bvb_sb = cf_sb[:, 4:260]
            # warmup source: the very first DVE op so the PE warmup
            # matmuls can start as soon as the engines come up
            wsrc = stage.tile([P, SB], BF16, tag="warm", name="wsrc")
            nc.vector.memset(wsrc[:], 0.5)
            # bias -3.5: max exp e^5.0 = 148 < fp8e4 max 240 (see module
            # docstring — bias -3 sat one fp8 step below the inf encoding).
            # NOT lower: typical exp values must stay in fp8 NORMAL range
            # (>= 2^-6); bias -4 pushed them subnormal (rel err 6e-3 -> 2e-2).
            expbias_sb = consts.tile([P, 1], F32)
            nc.vector.memset(expbias_sb[:], -3.5)

            # ---- x loads.  Each DMA trigger costs ~730ns of serial queue
            # time regardless of size, so x moves as ONE contiguous trigger
            # per 512-col block (the consumption granule), ALL on the sync
            # queue in exact need order (see module docstring: per-queue
            # fair-share bandwidth makes a second queue counterproductive).
            xqT = xT_pool.tile([P, NSLAB, DK, SB], BF16, tag="xT", name="xqT")
            xkT = xT_pool.tile([P, NSLAB, DK, SB], BF16, tag="xT", name="xkT")
            xv0T = xT_pool.tile([P, DK, SB], BF16, tag="xT", name="xv0T")
            xv8T = xT_pool.tile([P, NSLAB - 1, DK, SB], FP8, tag="xT",
                                name="xv8T")

            def ld(eng, xt, x_d, b):
                eng.dma_start(xt[:, b], x_d[b])

            # ONE queue, exact need order: the DMA engines split HBM
            # bandwidth fairly per QUEUE, so a second queue running early
            # non-critical transfers steals bandwidth from the critical
            # stream.  Within a queue, transfers complete in order at full
            # rate — need-order IS arrival-order.
            nc.sync.dma_start(wk_t[:], cb_d[:, 2048:4096])
            ld(nc.sync, xkT, xk_d, 0)
            nc.sync.dma_start(wq_t[:], cb_d[:, 0:2048])
            nc.sync.dma_start(cf_sb[:], cf_d)
            ld(nc.sync, xqT, xq_d, 3)
            nc.sync.dma_start(wv_t[:], cb_d[:, 4096:6144])
            nc.sync.dma_start(xv0T[:], xv0_d)
            nc.sync.dma_start(wv8_t[:], cb8_d)
            ld(nc.sync, xkT, xk_d, 1)
            nc.sync.dma_start(xv8T[:, 0], xv8_d[0])
            ld(nc.sync, xkT, xk_d, 2)
            nc.sync.dma_start(xv8T[:, 1], xv8_d[1])
            ld(nc.sync, xkT, xk_d, 3)
            nc.sync.dma_start(xv8T[:, 2], xv8_d[2])
            nc.sync.dma_start(wo_t[:], cb_d[:, 6144:8192])
            ld(nc.sync, xqT, xq_d, 2)
            ld(nc.sync, xqT, xq_d, 1)
            ld(nc.sync, xqT, xq_d, 0)

            qT_sb = acts.tile([P, C // P, S], BF16)       # [d_out, s] head-major
            kT_sb = acts.tile([P, C // P, S], BF16)
            headsT_sb = acts.tile([P, C // P, S], BF16)
            # v pair-indexed fp8: [p, kpair, h, i, col]; cols 0:64 v, 64:128
            # ones.  The h-major layout makes the DoubleRow lhsT slice
            # [:, kp, h, :, :] a contiguous trailing block (dependency
            # tracking of the interleaved [:, kp, :, h, :] form missed the
            # vproj writes -> first-execution read-before-write NaN).
            v_sb = acts.tile([P, KT // 2, HC, 2, P], FP8)
            nc.gpsimd.memset(v_sb[:, :, :, :, DH:P], 1.0)
            # bf16 copy of v k-tiles 0..3 for the all-bf16 slab-0 attention
            vbf_sb = acts.tile([P, 4, HC, P], BF16)
            nc.gpsimd.memset(vbf_sb[:, :, :, DH:P], 1.0)
            # causal strips generated on-device (saves 0.5MB of critical
            # startup DMA): strips[p, i, f] = 1.0 where p <= f - 128*i.
            # iota = f - 128*i - p; bf16 iota is inexact above |256| but the
            # comparison boundary (|iota| small) is exact.  gpsimd is the
            # only engine with affine_select; it is idle this early and
            # strips aren't consumed until slab 3's first diagonal pair.
            nc.gpsimd.memset(strips_t[:], 1.0)
            nc.gpsimd.affine_select(
                strips_t[:], strips_t[:],
                pattern=[[-P, SB // P], [1, SB]],
                compare_op=mybir.AluOpType.is_ge, fill=0.0,
                base=0, channel_multiplier=-1)

            def proj_unit(which, co, j):
                ps = pp.tile([P, SB], F32, tag="pp", name="proj_ps")
                if which == "q":
                    b_sb, outT = bq_sb, qT_sb
                    for o in range(DK):
                        nc.tensor.matmul(
                            ps[:],
                            lhsT=wq_sb[:, o, co * P:(co + 1) * P],
                            rhs=xqT[:, j, o, :],
                            start=(o == 0), stop=(o == DK - 1))
                else:
                    b_sb, outT = bk_sb, kT_sb
                    for o in range(DK):
                        nc.tensor.matmul(
                            ps[:],
                            lhsT=wk_sb[:, o, co * P:(co + 1) * P],
                            rhs=xkT[:, j, o, :],
                            start=(o == 0), stop=(o == DK - 1))
                nc.vector.tensor_scalar_add(
                    outT[:, co, j * SB:(j + 1) * SB], ps[:], b_sb[:, co:co + 1])

            def vproj_unit(st0, nst=2):
                for st in range(st0, st0 + nst):
                    ps = pp.tile([P, SB], F32, tag="pp", name="vproj_ps")
                    c0, c1 = (st % 4) * P, (st % 4 + 1) * P
                    if st < 4:
                        for o in range(DK):
                            nc.tensor.matmul(
                                ps[:, 0:C],
                                lhsT=xv0T[:, o, c0:c1],
                                rhs=wv_sb[:, o, :],
                                start=(o == 0), stop=(o == DK - 1))
                    else:
                        # fp8 x + fp8 w, o-chunk pairs fused via DoubleRow
                        for op in range(DK // 2):
                            nc.tensor.matmul(
                                ps[:, 0:C],
                                lhsT=xv8T[:, st // 4 - 1,
                                          2 * op:2 * op + 2, c0:c1],
                                rhs=wv8_sb[:, 2 * op:2 * op + 2, :],
                                start=(op == 0), stop=(op == DK // 2 - 1),
                                perf_mode=mybir.MatmulPerfMode.DoubleRow)
                    # one contiguous DVE add into a staging tile, then per-
                    # head gpsimd copies into v.  Per-head contiguous writes
                    # are required: a single strided (h-step 256) write is
                    # NOT seen as overlapping the DoubleRow lhsT read by the
                    # dependency tracker (first-exec read-before-write NaN);
                    # gpsimd does them so the DVE isn't op-count-bound.
                    vstage = stage.tile([P, C], BF16, tag="vst", name="vst")
                    nc.vector.tensor_add(vstage[:], ps[:, 0:C], bvb_sb[:])
                    for h in range(HC):
                        nc.gpsimd.tensor_copy(
                            v_sb[:, st // 2, h, st % 2, 0:DH],
                            vstage[:, h * DH:(h + 1) * DH])
                        if st < 4:
                            nc.gpsimd.tensor_copy(
                                vbf_sb[:, st, h, 0:DH],
                                vstage[:, h * DH:(h + 1) * DH])

            def oproj_unit(st, pool=None):
                ob = stage.tile([P, D], BF16, tag="ob", name="ob")
                if pool is None:
                    ps = {n2: pp.tile([P, SB], F32, tag="pp", name="o_ps")
                          for n2 in range(D // SB)}
                else:
                    # final units draw PSUM from the sps pool (free after the
                    # last exp): with pp's 2 bufs ALSO free this pipelines
                    # deeper instead of stalling on each unit's drains
                    pst = pool.tile([P, 2, SB], F32, tag="sps", name="o_ps2")
                    ps = {n2: pst[:, n2, :] for n2 in range(D // SB)}
                # cc-outer: both n2 matmuls of a cc share the stationary
                for cc in range(C // P):
                    for n2 in range(D // SB):
                        nc.tensor.matmul(
                            ps[n2][:],
                            lhsT=headsT_sb[:, cc, st * P:(st + 1) * P],
                            rhs=wo_sb[:, cc, n2 * SB:(n2 + 1) * SB],
                            start=(cc == 0), stop=(cc == C // P - 1))
                for n2 in range(D // SB):
                    # tail slabs (st<8 run during/after slab0): ACT is idle
                    # after the last exp while DVE still has normalize work —
                    # split the two PSUM->SBUF drains across both engines.
                    if st < 8 and n2 == 0:
                        nc.scalar.copy(ob[:, n2 * SB:(n2 + 1) * SB], ps[n2][:])
                    else:
                        nc.vector.tensor_copy(ob[:, n2 * SB:(n2 + 1) * SB],
                                              ps[n2][:])
                # stores go on the sync queue (idle once the input loads are
                # issued); a store trigger on the scalar queue would block
                # subsequent exp instructions — strict FIFO.  The final units
                # (sps pool) store per half so the first half streams out
                # while the second is still copying.
                if pool is None:
                    nc.sync.dma_start(o_d[st * P:(st + 1) * P, :], ob[:])
                else:
                    for n2 in range(D // SB):
                        nc.sync.dma_start(
                            o_d[st * P:(st + 1) * P, n2 * SB:(n2 + 1) * SB],
                            ob[:, n2 * SB:(n2 + 1) * SB])
                if dump and st == 12:
                    nc.sync.dma_start(ob_o, ob[:])

            def attn_slab(j, fillers=None):
                # fillers: {pair_index: [closures]}.  Tile derives
                # dependencies from EMISSION order (a reader only waits on
                # earlier-emitted writers), so every filler must be emitted
                # before its first consumer — never after it.
                fillers = dict(fillers or {})
                n_kt = 4 * (j + 1) if causal else KT
                bfslab = causal and j == 0    # slab 0 all-bf16
                edt = BF16 if bfslab else FP8
                etag = "expTbf" if bfslab else "expT"
                it = 0
                for hc in range(HC // 2):          # head pair (2hc, 2hc+1)
                    outp = [otp.tile([P, SB], F32, tag="otp", name=f"outp{hh}")
                            for hh in range(2)]
                    # two-deep pending P@V queue: P@V(tb) executes two
                    # iterations after its scores, giving the xv8 DMA an
                    # extra iteration of arrival slack in slab 3 (accumulate
                    # order into outp is preserved)
                    pend = [[], []]

                    def qlo(t):   # valid-q offset within slab for k-tile t
                        if not causal:
                            return 0
                        return max(0, P * t - SB * j)

                    def make_pav(tb, qp, hh, expT_t):
                        # expT_t is the iteration's [P, 2(d), 2(hh), SB]
                        # tile; head hh's k-tile pair is the strided slice
                        # [:, :, hh, :]
                        h = 2 * hc + hh

                        def pav():
                            if bfslab:
                                for d_ in range(2):
                                    t = tb + d_
                                    ql = qlo(t)
                                    nc.tensor.matmul(
                                        outp[hh][:, ql:],
                                        lhsT=vbf_sb[:, t, h, :],
                                        rhs=expT_t[:, d_, hh, ql:],
                                        start=(t == 0), stop=(t == n_kt - 1))
                            elif (causal and tb >= 4 * j) or \
                                    mask_mode == "generic":
                                # masked pairs run as 2 plain fp8 matmuls:
                                # their [d_, hh, qp:] rhs reads structurally
                                # match the strip/mask tensor_mul writes, so
                                # the dependency tracker sees them
                                for d_ in range(2):
                                    nc.tensor.matmul(
                                        outp[hh][:, qp:],
                                        lhsT=v_sb[:, tb // 2, h, d_, :],
                                        rhs=expT_t[:, d_, hh, qp:],
                                        start=(tb == 0 and d_ == 0),
                                        stop=(d_ == 1 and tb == n_kt - 2))
                            else:
                                # fused P@V: one DoubleRow matmul per k-tile
                                # pair (2 fp8 weights/cell; virtual K=256)
                                nc.tensor.matmul(
                                    outp[hh][:, qp:],
                                    lhsT=v_sb[:, tb // 2, h, :, :],
                                    rhs=expT_t[:, :, hh, qp:],
                                    start=(tb == 0), stop=(tb == n_kt - 2),
                                    perf_mode=mybir.MatmulPerfMode.DoubleRow)
                        return pav

                    # d-major iteration: one [P,2,SB] PSUM tile per k-tile of
                    # the pair, each holding BOTH heads' scores (bank=hh).
                    # The 2 MMs of a d become READY together (single WAR edge
                    # on that d's exp), so the scheduler keeps them adjacent
                    # and — being different row groups (h0/h64) — they run
                    # CONCURRENT in the array.  (With per-hh tiles, readiness
                    # was staggered by the previous iteration's exps and the
                    # scheduler regrouped the MMs per row group — no
                    # concurrency.  With one merged 4-bank tile, scores(i+1)
                    # had to wait the FULL exp(i) — 1.5us PE stalls and HAM
                    # re-throttle.)  exp runs per d over both heads (N=1024);
                    # expT is [P, 2(d), 2(hh), SB] so the DR P@V rhs is the
                    # clean strided slice [:, :, hh, qp:].
                    for tb in range(0, n_kt, 2):
                        myfill = fillers.pop(it, ())
                        it += 1
                        qb = qlo(tb)
                        qp = qb
                        mloads = []
                        # both tiles write from the PAIR base qb: exp reads
                        # [qb:], and a never-written stale PSUM value there
                        # can exp() to inf -> fp8 INF -> NaN via the
                        # strip-zero multiply.
                        spd = [sps.tile([P, 2, SB], F32, tag="sps", name="sp")
                               for _ in range(2)]
                        expT4 = exp_pool.tile([P, 2, 2, SB], edt, tag=etag,
                                              name="expT")
                        for d_ in range(2):
                            t = tb + d_
                            for hh in range(2):
                                hp = DH * hh
                                nc.tensor.matmul(
                                    spd[d_][:, hh, qb:],
                                    lhsT=kT_sb[hp:hp + DH, hc,
                                               t * P:(t + 1) * P],
                                    rhs=qT_sb[hp:hp + DH, hc,
                                              j * SB + qb:(j + 1) * SB],
                                    start=True, stop=True,
                                    tile_position=(hp, 0))
                        # bias -3.5: keeps exp within fp8e4 range (softmax
                        # is shift-invariant; numerator and denominator
                        # share the e^-3.5 factor)
                        for d_ in range(2):
                            nc.scalar.activation(
                                expT4[:, d_, :, qb:],
                                spd[d_][:, 0:2, qb:], AFT.Exp, scale=SCALE,
                                bias=expbias_sb[:])
                        if dump and j == 3 and hc == 0 and not bfslab:
                            for hh in range(2):
                                nc.sync.dma_start(
                                    expT_o[:, hh, tb // 2, :, :],
                                    expT4[:, :, hh, :])
                        # pending P@Vs go BEFORE the fillers: they are always
                        # data-ready (their v tiles came from earlier
                        # iterations), while a filler stuck on a DMA arrival
                        # would block them in the PE FIFO and idle the array
                        # long enough to re-throttle the clock
                        for hh in range(2):
                            if len(pend[hh]) >= 2:
                                pend[hh].pop(0)()
                        # fillers after this pair's scores (so they are not
                        # queued ahead of them) but before the P@V that may
                        # consume their output
                        for f in myfill:
                            f()
                        for d_ in range(2):
                            t = tb + d_
                            if mask_mode == "generic":
                                m_sb = stage.tile([P, SB], BF16, tag="msk",
                                                  name="m_sb")
                                nc.gpsimd.dma_start(
                                    m_sb[:],
                                    maskT_d[t * P:(t + 1) * P,
                                            j * SB:(j + 1) * SB])
                                mloads.append(m_sb)
                            if mask_mode == "generic":
                                for hh in range(2):
                                    nc.vector.tensor_mul(
                                        expT4[:, d_, hh, :],
                                        expT4[:, d_, hh, :],
                                        mloads[d_][:])
                            elif causal and t >= 4 * j:
                                # mask relative to the PAIR's q window:
                                # d_=0: triangle on 128 cols; d_=1: zero
                                # prefix + triangle over 256 cols; one mul
                                # covers BOTH heads' banks of this d
                                w = min((d_ + 1) * P, SB - qp)
                                for hh in range(2):
                                    nc.vector.tensor_mul(
                                        expT4[:, d_, hh, qp:qp + w],
                                        expT4[:, d_, hh, qp:qp + w],
                                        strips_sb[:, d_, 0:w])
                        for hh in range(2):
                            pend[hh].append(make_pav(tb, qp, hh, expT4))
                    for hh in range(2):
                        for f in pend[hh]:
                            f()
                    if dump and j == 3 and hc == 0:
                        for hh in range(2):
                            oc = stage.tile([P, SB], F32, tag="recip",
                                            name="oc_dump")
                            nc.vector.tensor_copy(oc[:], outp[hh][:])
                            nc.sync.dma_start(outp_o[:, hh, 0, :], oc[:])
                    # normalize: rows 64:128 of outp hold the denominator.
                    # reciprocal_approx_fast mis-reads when in/out partition
                    # bases differ, so run it over all 128 rows (rows 0:64
                    # are recip of the unnormalized output — unused).
                    for hh in range(2):
                        hp = DH * hh
                        recip = stage.tile([P, SB], F32, tag="recip",
                                           name="recip")
                        nc.vector.reciprocal_approx_fast(recip[:], outp[hh][:])
                        nc.vector.tensor_mul(
                            headsT_sb[hp:hp + DH, hc, j * SB:(j + 1) * SB],
                            outp[hh][0:DH, :], recip[DH:P, :])
                for k in sorted(fillers):
                    for f in fillers[k]:
                        f()

            def F(fn, *a):
                return lambda: fn(*a)

            # ---- schedule: slabs in descending work order; projection and
            # output-projection units woven into attention iterations so the
            # PE stays dense while ACT (exp) is the bottleneck.
            # PE warmup: dummy matmuls run while the first DMAs land and
            # flip the HAM clock gate to 2.4 GHz before real work arrives
            def ka(n, w=P):
                # keep-alive matmuls: no data deps, so they run the moment
                # the PE FIFO reaches them.  A short (N=128) pulse is enough
                # to break the HAM idle window during a DMA-arrival stall;
                # only the initial warm-up burst needs sustained busy (w=SB).
                for i in range(n):
                    wps = pp.tile([P, SB], F32, tag="pp", name="warm_ps")
                    nc.tensor.matmul(wps[:, 0:w], lhsT=wsrc[:, 0:P],
                                     rhs=wsrc[:, 0:w], start=True, stop=True)

            ka(12, w=SB)
            proj_unit("k", 0, 0)
            ka(8, w=SB)
            proj_unit("q", 0, 3)
            # Filler placement rule: each unit is emitted at (or before) the
            # iteration whose instructions first consume its output —
            # scores(hc, tb) need kT/qT of j-slab tb//4, P@V(hc0, tb) needs
            # v tiles tb..tb+1 (iteration index tb//2).
            attn_slab(3, {
                0: [F(ka, 1), F(vproj_unit, 0)],
                1: [F(ka, 1), F(vproj_unit, 2), F(proj_unit, "k", 0, 1),
                    F(proj_unit, "q", 1, 3)],
                2: [F(ka, 1), F(vproj_unit, 4)],
                3: [F(ka, 1), F(vproj_unit, 6), F(proj_unit, "k", 0, 2)],
                4: [F(ka, 1), F(vproj_unit, 8)],
                5: [F(ka, 1), F(vproj_unit, 10), F(proj_unit, "k", 0, 3),
                    F(proj_unit, "k", 1, 0)],
                6: [F(ka, 1), F(vproj_unit, 12)],
                7: [F(ka, 1), F(vproj_unit, 14)],
                9: [F(proj_unit, "k", 1, 1), F(proj_unit, "q", 0, 2)],
                11: [F(proj_unit, "k", 1, 2)],
                13: [F(proj_unit, "k", 1, 3)],
            })
            attn_slab(2, {
                0: [F(oproj_unit, 12)],
                1: [F(oproj_unit, 13)],
                2: [F(oproj_unit, 14), F(proj_unit, "q", 1, 2)],
                3: [F(oproj_unit, 15)],
                6: [F(proj_unit, "q", 0, 1)],
                8: [F(proj_unit, "q", 1, 1)],
            })
            attn_slab(1, {
                0: [F(oproj_unit, 8)], 1: [F(oproj_unit, 9)],
                2: [F(oproj_unit, 10)], 3: [F(oproj_unit, 11)],
                5: [F(proj_unit, "q", 0, 0)],
                6: [F(proj_unit, "q", 1, 0)],
            })
            attn_slab(0, {
                0: [F(ka, 1), F(oproj_unit, 4)],
                1: [F(ka, 1), F(oproj_unit, 5)],
                2: [F(ka, 1), F(oproj_unit, 6)],
                3: [F(ka, 1), F(oproj_unit, 7)],
            })
            for st in range(4):
                ka(1)
                oproj_unit(st, pool=sps)
            ka(2)

            if dump:
                nc.scalar.dma_start(qT_o, qT_sb[:])
                nc.scalar.dma_start(kT_o, kT_sb[:])
                nc.scalar.dma_start(v_o, v_sb[:])
                nc.scalar.dma_start(vbf_o, vbf_sb[:])
                nc.scalar.dma_start(heads_o, headsT_sb[:])
                nc.scalar.dma_start(strips_o, strips_sb[:])

    nc.compile()
    return nc


@functools.lru_cache(maxsize=4)
def _get(mask_mode: str):
    return _build(mask_mode)


def _bf16(a):
    return np.ascontiguousarray(a.astype(ml_dtypes.bfloat16))


def _detect_mask_mode(m):
    if (m == 1).all():
        return "none"
    idx = np.arange(m.shape[0])
    if np.array_equal(m != 0, idx[None, :] <= idx[:, None]):
        return "causal"
    return "generic"


def _strips():
    p = np.arange(P)[:, None]
    f = np.arange(SB)[None, :]
    s = np.stack([(p <= f - P * i) for i in range(SB // P)], axis=1)
    return np.ascontiguousarray(s.astype(ml_dtypes.bfloat16))


def prepare(query, key, value, mask, Wq, bq, Wk, bk, Wv, bv, Wo, bo):
    """Returns (mask_mode, in_maps) for run_bass_kernel_spmd."""
    query = np.asarray(query, dtype=np.float32)
    key = np.asarray(key, dtype=np.float32)
    value = np.asarray(value, dtype=np.float32)
    m2d = np.asarray(mask).reshape(np.asarray(mask).shape[-2:])
    mask_mode = _detect_mask_mode(m2d)

    def prep_x(x):    # [S, D] -> 512-blocked transposed [NSLAB, P, DK*SB] f32
        # xT[b, p, o*SB + c] = x[SB*b + c, P*o + p]
        xt = np.asarray(x, np.float32).T.reshape(DK, P, NSLAB, SB)
        return np.ascontiguousarray(
            xt.transpose(2, 1, 0, 3).reshape(NSLAB, P, DK * SB))

    def _fp8(a):
        return np.ascontiguousarray(a.astype(ml_dtypes.float8_e4m3))

    xq = [_bf16(prep_x(query[b])) for b in range(B)]
    xk = [_bf16(prep_x(key[b])) for b in range(B)]
    xvf = [prep_x(value[b]) for b in range(B)]
    xv0 = [_bf16(xvf[b][0]) for b in range(B)]
    xv8 = [_fp8(xvf[b][1:]) for b in range(B)]

    def prep_w(W, g):     # rows [256g, 256g+256) of W, transposed -> [128, 8, 256]
        sl = np.asarray(W, np.float32)[g * C:(g + 1) * C, :].T
        return _bf16(sl.reshape(DK, P, C).transpose(1, 0, 2))

    def prep_wo(g):       # Wo[:, 256g:256g+256].T -> [128, 2, 1024]
        sl = np.asarray(Wo, np.float32)[:, g * C:(g + 1) * C].T
        return _bf16(sl.reshape(C // P, P, D).transpose(1, 0, 2))

    def prep_b(b_, g):
        sl = np.asarray(b_, np.float32)[g * C:(g + 1) * C]
        return np.ascontiguousarray(sl.reshape(C // P, P).T)

    def prep_bvb(g):
        sl = np.asarray(bv, np.float32)[g * C:(g + 1) * C]
        return np.ascontiguousarray(np.broadcast_to(sl[None, :], (P, C)))

    strips = _strips()
    maskT = _bf16(m2d.T.astype(np.float32)) if mask_mode == "generic" else None

    in_maps = []
    for c in range(NCORES):
        b, g = c // GROUPS, c % GROUPS
        cb = np.concatenate([
            prep_w(Wq, g).reshape(P, 2048), prep_w(Wk, g).reshape(P, 2048),
            prep_w(Wv, g).reshape(P, 2048), prep_wo(g).reshape(P, 2048),
            strips.reshape(P, 2048)], axis=1)
        cf = np.concatenate([
            prep_b(bq, g), prep_b(bk, g), prep_bvb(g)], axis=1)
        def prep_w8(W):
            return (np.asarray(W, np.float32)[g * C:(g + 1) * C, :].T
                    .reshape(DK, P, C).transpose(1, 0, 2).reshape(P, DK * C))

        im = dict(xq=xq[b], xk=xk[b], xv0=xv0[b], xv8=xv8[b],
                  cb=np.ascontiguousarray(cb),
                  cb8=_fp8(prep_w8(Wv)),
                  cf=np.ascontiguousarray(cf.astype(np.float32)))
        if maskT is not None:
            im["maskT"] = maskT
        in_maps.append(im)

    return mask_mode, in_maps


def kernel(query, key, value, mask, Wq, bq, Wk, bk, Wv, bv, Wo, bo):
    mask_mode, in_maps = prepare(query, key, value, mask, Wq, bq, Wk, bk,
                                 Wv, bv, Wo, bo)
    nc = _get(mask_mode)
    res = run_bass_kernel_spmd(nc, in_maps, list(range(NCORES)))
    partials = np.stack([res.results[c]["o"].astype(np.float32)
                         for c in range(NCORES)])
    out = partials.reshape(B, GROUPS, S, D).sum(axis=1)
    out = out + np.asarray(bo, np.float32)[None, None, :]
    return out.astype(np.float32)



# revision 84
# speedup vs baseline: 1.0198x; 1.0013x over previous
"""Multi-head attention (B=2, S=2048, D=1024, H=16, causal) on 8 Trainium2 cores.

Sharding: core c handles batch b = c // 4 and head group g = c % 4 (4 heads,
d_model column slice [256*g, 256*g+256)).  QKV projections are computed per
core against the full sequence of its batch; attention runs per head in a
"scores-transposed" [k, q] layout; the output projection produces a per-core
partial [S, D] (bf16) that the host sums over the 4 head-group cores.

v5 perf structure (PE-bound; ~160us on 8 cores):
- x is transposed AND 512-col-blocked on the HOST to [NSLAB, 128, DK*SB]:
  each block is ONE fully-contiguous [128 x 8KB-row] DMA.  DMA triggers
  cost ~730ns of serial queue time each regardless of size, and the DMA
  engines split HBM bandwidth fairly per QUEUE — so ALL input loads go on
  the sync queue as few big triggers in exact need order (arrival order ==
  need order at full 358 GB/s).
- Scores use d-major PSUM tiles (one [P,2,SB] tile per k-tile of a pair,
  banks = heads) with exp per d over both heads: the two 64-row score MMs
  of a d become READY together, so the Tile scheduler keeps them adjacent
  and they run CONCURRENT in alternating PE row groups (h0/h64) — measured
  ~82% of adjacent score MMs start within 6ns.  (Per-hh tiles stagger
  readiness by the previous iteration's exps and the scheduler regroups MMs
  per row group: no concurrency.  One merged 4-bank tile forces scores(i+1)
  to wait the full exp(i): 1.5us PE stalls + HAM re-throttle.)
- P@V runs in fp8e4 DoubleRow perf mode: expT ([P, 2, SB]) and v
  ([P, 2, HC, P]) are pair-indexed along k-tiles; each k-tile PAIR is one
  fused virtual-K=256 matmul at 2x bf16 throughput.  exp writes fp8 directly
  (bias -3.5: device fp8e4 is IEEE e4m3, max 240 WITH an inf encoding; max
  raw causal score ~68 -> exp arg 68/8-3.5=5.0 -> e^5=148 < 240.  At the
  original bias -3 the max exp was 237 — one fp8 step from inf, and an inf
  exp makes outp/denominator inf -> NaN output rows).  The softmax
  denominator comes from 64 ones-columns appended to V.
- The V projection for s-tiles 4-15 runs fp8 DoubleRow (x and Wv fp8 from
  host, o-chunk pairs fused); tiles 0-3 stay bf16 — they feed the all-bf16
  slab-0 attention (fp8 error concentrates in early rows where softmax
  averages few values).  Q/K projections MUST stay bf16: fp8 k alone costs
  ~2.3e-2 rel err (score noise hits early-row softmax hard) vs the 2e-2
  gate.
- Slabs run in DESCENDING order (3,2,1,0); projection / output-projection
  units are WOVEN into the attention iterations as PE filler.  The pending
  P@V is flushed BEFORE each iteration's fillers (it is always data-ready;
  a filler stuck on a DMA arrival would block it in the PE FIFO).
  Keep-alive matmuls (no data deps) bridge DMA-arrival gaps in the lead-in
  and at iteration starts so the HAM clock gate never sees a full idle
  window (which would re-throttle the PE to 1.2 GHz).
- Final 4 output-projection units draw PSUM from the post-exp-idle sps pool
  (4-deep pipelining with pp) and store per 512-half for an earlier finish.
"""

import functools
import os
import numpy as np
import ml_dtypes

import concourse.bass as bass
import concourse.bacc as bacc
import concourse.tile as tile
import concourse.mybir as mybir
from concourse.bass_utils import run_bass_kernel_spmd

dt = mybir.dt
F32 = dt.float32
BF16 = dt.bfloat16
FP8 = dt.float8e4
AFT = mybir.ActivationFunctionType

B, S, D = 2, 2048, 1024
H, DH = 16, 64
NCORES = 8
GROUPS = NCORES // B            # 4 head-groups
HC = H // GROUPS                # 4 heads per core
C = HC * DH                     # 256 = per-core head-column slice
P = 128
DK = D // P                     # 8 d_in chunks
SB = 512                        # q-slab width
NSLAB = S // SB                 # 4
KT = S // P                     # 16 k tiles
SCALE = 1.0 / float(np.sqrt(DH))


def _build(mask_mode: str):
    """mask_mode: 'causal' | 'none' | 'generic'. Returns compiled Bacc."""
    assert mask_mode in ("causal", "none", "generic")
    causal = mask_mode == "causal"
    nc = bacc.Bacc("TRN2", target_bir_lowering=False, debug=False)

    # host-transposed x, 512-col blocked: xT[b, p, o, c] = x[512*b + c, 128*o + p]
    # -> each 512-block is ONE fully-contiguous [128 x 4096] DMA (one trigger).
    xq_d = nc.dram_tensor("xq", [NSLAB, P, DK * SB], BF16, kind="ExternalInput").ap()
    xk_d = nc.dram_tensor("xk", [NSLAB, P, DK * SB], BF16, kind="ExternalInput").ap()
    # xv block 0 stays bf16 (feeds the all-bf16 slab-0 v); blocks 1-3 are fp8
    # so their vproj runs DoubleRow (v is quantized to fp8 for P@V anyway)
    xv0_d = nc.dram_tensor("xv0", [P, DK * SB], BF16, kind="ExternalInput").ap()
    xv8_d = nc.dram_tensor("xv8", [NSLAB - 1, P, DK * SB], FP8,
                           kind="ExternalInput").ap()
    # all bf16 constants packed into one tensor: wq|wk|wv|wo|strips
    cb_d = nc.dram_tensor("cb", [P, 5 * 2048], BF16, kind="ExternalInput").ap()
    cb8_d = nc.dram_tensor("cb8", [P, 2048], FP8, kind="ExternalInput").ap()
    cf_d = nc.dram_tensor("cf", [P, 260], F32, kind="ExternalInput").ap()
    if mask_mode == "generic":
        maskT_d = nc.dram_tensor("maskT", [S, S], BF16, kind="ExternalInput").ap()
    o_d = nc.dram_tensor("o", [S, D], BF16, kind="ExternalOutput").ap()
    dump = os.environ.get("K_DUMP") == "1"
    if dump:
        qT_o = nc.dram_tensor("qT_o", [P, C // P, S], BF16, kind="ExternalOutput").ap()
        kT_o = nc.dram_tensor("kT_o", [P, C // P, S], BF16, kind="ExternalOutput").ap()
        v_o = nc.dram_tensor("v_o", [P, KT // 2, 2, HC, P], FP8, kind="ExternalOutput").ap()
        vbf_o = nc.dram_tensor("vbf_o", [P, 4, HC, P], BF16, kind="ExternalOutput").ap()
        heads_o = nc.dram_tensor("heads_o", [P, C // P, S], BF16, kind="ExternalOutput").ap()
        strips_o = nc.dram_tensor("strips_o", [P, SB // P, SB], BF16, kind="ExternalOutput").ap()
        expT_o = nc.dram_tensor("expT_o", [P, 2, 16, 2, SB], FP8, kind="ExternalOutput").ap()
        spc_o = nc.dram_tensor("spc_o", [P, 8, 2, SB], F32, kind="ExternalOutput").ap()
        outp_o = nc.dram_tensor("outp_o", [P, 2, 2, SB], F32, kind="ExternalOutput").ap()
        ob_o = nc.dram_tensor("ob_o", [P, D], BF16, kind="ExternalOutput").ap()

    with tile.TileContext(nc) as tc:
        with (
            tc.tile_pool(name="consts", bufs=1) as consts,
            tc.tile_pool(name="xT", bufs=3) as xT_pool,
            tc.tile_pool(name="acts", bufs=1) as acts,
            tc.tile_pool(name="expT", bufs=4) as exp_pool,
            tc.tile_pool(name="stage", bufs=2) as stage,
            tc.tile_pool(name="pp", bufs=2, space="PSUM") as pp,
            tc.tile_pool(name="sps", bufs=2, space="PSUM") as sps,
            tc.tile_pool(name="otp", bufs=2, space="PSUM") as otp,
        ):
            # ---- constants: one dedicated tile per constant, each written by
            # exactly one whole-tile DMA (readers then depend on the whole
            # write — no sub-range matching through view slices).
            wk_t = consts.tile([P, DK, C], BF16, name="wk")
            wq_t = consts.tile([P, DK, C], BF16, name="wq")
            wv_t = consts.tile([P, DK, C], BF16, name="wv")
            wv8_t = consts.tile([P, DK, C], FP8, name="wv8")
            wo_t = consts.tile([P, C // P, D], BF16, name="wo")
            strips_t = consts.tile([P, SB // P, SB], BF16, name="strips")
            cf_sb = consts.tile([P, 260], F32)
            # weights go FIRST on the sync HWDGE ring (SWDGE is slow to
            # start and wk gates the very first projection)
            wq_sb, wk_sb, wv_sb = wq_t[:], wk_t[:], wv_t[:]
            wv8_sb = wv8_t[:]
            wo_sb, strips_sb = wo_t[:], strips_t[:]
            bq_sb = cf_sb[:, 0:2]
            bk_sb = cf_sb[:, 2:4]
            bvb_sb = cf_sb[:, 4:260]
            # warmup source: the very first DVE op so the PE warmup
            # matmuls can start as soon as the engines come up
            wsrc = stage.tile([P, SB], BF16, tag="warm", name="wsrc")
            nc.vector.memset(wsrc[:], 0.5)
            # bias -3.5: max exp e^5.0 = 148 < fp8e4 max 240 (see module
            # docstring — bias -3 sat one fp8 step below the inf encoding).
            # NOT lower: typical exp values must stay in fp8 NORMAL range
            # (>= 2^-6); bias -4 pushed them subnormal (rel err 6e-3 -> 2e-2).
            expbias_sb = consts.tile([P, 1], F32)
            nc.vector.memset(expbias_sb[:], -3.5)

            # ---- x loads.  Each DMA trigger costs ~730ns of serial queue
            # time regardless of size, so x moves as ONE contiguous trigger
            # per 512-col block (the consumption granule), ALL on the sync
            # queue in exact need order (see module docstring: per-queue
            # fair-share bandwidth makes a second queue counterproductive).
            xqT = xT_pool.tile([P, NSLAB, DK, SB], BF16, tag="xT", name="xqT")
            xkT = xT_pool.tile([P, NSLAB, DK, SB], BF16, tag="xT", name="xkT")
            xv0T = xT_pool.tile([P, DK, SB], BF16, tag="xT", name="xv0T")
            xv8T = xT_pool.tile([P, NSLAB - 1, DK, SB], FP8, tag="xT",
                                name="xv8T")

            def ld(eng, xt, x_d, b):
                eng.dma_start(xt[:, b], x_d[b])

            # ONE queue, exact need order: the DMA engines split HBM
            # bandwidth fairly per QUEUE, so a second queue running early
            # non-critical transfers steals bandwidth from the critical
            # stream.  Within a queue, transfers complete in order at full
            # rate — need-order IS arrival-order.
            nc.sync.dma_start(wk_t[:], cb_d[:, 2048:4096])
            ld(nc.sync, xkT, xk_d, 0)
            nc.sync.dma_start(wq_t[:], cb_d[:, 0:2048])
            nc.sync.dma_start(cf_sb[:], cf_d)
            ld(nc.sync, xqT, xq_d, 3)
            nc.sync.dma_start(wv_t[:], cb_d[:, 4096:6144])
            nc.sync.dma_start(xv0T[:], xv0_d)
            nc.sync.dma_start(wv8_t[:], cb8_d)
            ld(nc.sync, xkT, xk_d, 1)
            nc.sync.dma_start(xv8T[:, 0], xv8_d[0])
            ld(nc.sync, xkT, xk_d, 2)
            nc.sync.dma_start(xv8T[:, 1], xv8_d[1])
            ld(nc.sync, xkT, xk_d, 3)
            nc.sync.dma_start(xv8T[:, 2], xv8_d[2])
            nc.sync.dma_start(wo_t[:], cb_d[:, 6144:8192])
            ld(nc.sync, xqT, xq_d, 2)
            ld(nc.sync, xqT, xq_d, 1)
            ld(nc.sync, xqT, xq_d, 0)

            qT_sb = acts.tile([P, C // P, S], BF16)       # [d_out, s] head-major
            kT_sb = acts.tile([P, C // P, S], BF16)
            headsT_sb = acts.tile([P, C // P, S], BF16)
            # v pair-indexed fp8: [p, kpair, h, i, col]; cols 0:64 v, 64:128
            # ones.  The h-major layout makes the DoubleRow lhsT slice
            # [:, kp, h, :, :] a contiguous trailing block (dependency
            # tracking of the interleaved [:, kp, :, h, :] form missed the
            # vproj writes -> first-execution read-before-write NaN).
            v_sb = acts.tile([P, KT // 2, HC, 2, P], FP8)
            nc.gpsimd.memset(v_sb[:, :, :, :, DH:P], 1.0)
            # bf16 copy of v k-tiles 0..3 for the all-bf16 slab-0 attention
            vbf_sb = acts.tile([P, 4, HC, P], BF16)
            nc.gpsimd.memset(vbf_sb[:, :, :, DH:P], 1.0)
            # causal strips generated on-device (saves 0.5MB of critical
            # startup DMA): strips[p, i, f] = 1.0 where p <= f - 128*i.
            # iota = f - 128*i - p; bf16 iota is inexact above |256| but the
            # comparison boundary (|iota| small) is exact.  gpsimd is the
            # only engine with affine_select; it is idle this early and
            # strips aren't consumed until slab 3's first diagonal pair.
            nc.gpsimd.memset(strips_t[:], 1.0)
            nc.gpsimd.affine_select(
                strips_t[:], strips_t[:],
                pattern=[[-P, SB // P], [1, SB]],
                compare_op=mybir.AluOpType.is_ge, fill=0.0,
                base=0, channel_multiplier=-1)

            def proj_unit(which, co, j):
                ps = pp.tile([P, SB], F32, tag="pp", name="proj_ps")
                if which == "q":
                    b_sb, outT = bq_sb, qT_sb
                    for o in range(DK):
                        nc.tensor.matmul(
                            ps[:],
                            lhsT=wq_sb[:, o, co * P:(co + 1) * P],
                            rhs=xqT[:, j, o, :],
                            start=(o == 0), stop=(o == DK - 1))
                else:
                    b_sb, outT = bk_sb, kT_sb
                    for o in range(DK):
                        nc.tensor.matmul(
                            ps[:],
                            lhsT=wk_sb[:, o, co * P:(co + 1) * P],
                            rhs=xkT[:, j, o, :],
                            start=(o == 0), stop=(o == DK - 1))
                nc.vector.tensor_scalar_add(
                    outT[:, co, j * SB:(j + 1) * SB], ps[:], b_sb[:, co:co + 1])

            def vproj_unit(st0, nst=2):
                for st in range(st0, st0 + nst):
                    ps = pp.tile([P, SB], F32, tag="pp", name="vproj_ps")
                    c0, c1 = (st % 4) * P, (st % 4 + 1) * P
                    if st < 4:
                        for o in range(DK):
                            nc.tensor.matmul(
                                ps[:, 0:C],
                                lhsT=xv0T[:, o, c0:c1],
                                rhs=wv_sb[:, o, :],
                                start=(o == 0), stop=(o == DK - 1))
                    else:
                        # fp8 x + fp8 w, o-chunk pairs fused via DoubleRow
                        for op in range(DK // 2):
                            nc.tensor.matmul(
                                ps[:, 0:C],
                                lhsT=xv8T[:, st // 4 - 1,
                                          2 * op:2 * op + 2, c0:c1],
                                rhs=wv8_sb[:, 2 * op:2 * op + 2, :],
                                start=(op == 0), stop=(op == DK // 2 - 1),
                                perf_mode=mybir.MatmulPerfMode.DoubleRow)
                    # one contiguous DVE add into a staging tile, then per-
                    # head gpsimd copies into v.  Per-head contiguous writes
                    # are required: a single strided (h-step 256) write is
                    # NOT seen as overlapping the DoubleRow lhsT read by the
                    # dependency tracker (first-exec read-before-write NaN);
                    # gpsimd does them so the DVE isn't op-count-bound.
                    vstage = stage.tile([P, C], BF16, tag="vst", name="vst")
                    nc.vector.tensor_add(vstage[:], ps[:, 0:C], bvb_sb[:])
                    for h in range(HC):
                        nc.gpsimd.tensor_copy(
                            v_sb[:, st // 2, h, st % 2, 0:DH],
                            vstage[:, h * DH:(h + 1) * DH])
                        if st < 4:
                            nc.gpsimd.tensor_copy(
                                vbf_sb[:, st, h, 0:DH],
                                vstage[:, h * DH:(h + 1) * DH])

            def oproj_unit(st, pool=None):
                ob = stage.tile([P, D], BF16, tag="ob", name="ob")
                if pool is None:
                    ps = {n2: pp.tile([P, SB], F32, tag="pp", name="o_ps")
                          for n2 in range(D // SB)}
                else:
                    # final units draw PSUM from the sps pool (free after the
                    # last exp): with pp's 2 bufs ALSO free this pipelines
                    # deeper instead of stalling on each unit's drains
                    pst = pool.tile([P, 2, SB], F32, tag="sps", name="o_ps2")
                    ps = {n2: pst[:, n2, :] for n2 in range(D // SB)}
                # cc-outer: both n2 matmuls of a cc share the stationary
                for cc in range(C // P):
                    for n2 in range(D // SB):
                        nc.tensor.matmul(
                            ps[n2][:],
                            lhsT=headsT_sb[:, cc, st * P:(st + 1) * P],
                            rhs=wo_sb[:, cc, n2 * SB:(n2 + 1) * SB],
                            start=(cc == 0), stop=(cc == C // P - 1))
                for n2 in range(D // SB):
                    # tail slabs (st<8 run during/after slab0): ACT is idle
                    # after the last exp while DVE still has normalize work —
                    # split the two PSUM->SBUF drains across both engines.
                    if st < 8 and n2 == 0:
                        nc.scalar.copy(ob[:, n2 * SB:(n2 + 1) * SB], ps[n2][:])
                    else:
                        nc.vector.tensor_copy(ob[:, n2 * SB:(n2 + 1) * SB],
                                              ps[n2][:])
                # stores go on the sync queue (idle once the input loads are
                # issued); a store trigger on the scalar queue would block
                # subsequent exp instructions — strict FIFO.  The final units
                # (sps pool) store per half so the first half streams out
                # while the second is still copying.
                if pool is None:
                    nc.sync.dma_start(o_d[st * P:(st + 1) * P, :], ob[:])
                else:
                    for n2 in range(D // SB):
                        nc.sync.dma_start(
                            o_d[st * P:(st + 1) * P, n2 * SB:(n2 + 1) * SB],
                            ob[:, n2 * SB:(n2 + 1) * SB])
                if dump and st == 12:
                    nc.sync.dma_start(ob_o, ob[:])

            def attn_slab(j, fillers=None):
                # fillers: {pair_index: [closures]}.  Tile derives
                # dependencies from EMISSION order (a reader only waits on
                # earlier-emitted writers), so every filler must be emitted
                # before its first consumer — never after it.
                fillers = dict(fillers or {})
                n_kt = 4 * (j + 1) if causal else KT
                bfslab = causal and j == 0    # slab 0 all-bf16
                edt = BF16 if bfslab else FP8
                etag = "expTbf" if bfslab else "expT"
                it = 0
                for hc in range(HC // 2):          # head pair (2hc, 2hc+1)
                    outp = [otp.tile([P, SB], F32, tag="otp", name=f"outp{hh}")
                            for hh in range(2)]
                    # two-deep pending P@V queue: P@V(tb) executes two
                    # iterations after its scores, giving the xv8 DMA an
                    # extra iteration of arrival slack in slab 3 (accumulate
                    # order into outp is preserved)
                    pend = [[], []]

                    def qlo(t):   # valid-q offset within slab for k-tile t
                        if not causal:
                            return 0
                        return max(0, P * t - SB * j)

                    def make_pav(tb, qp, hh, expT_t):
                        # expT_t is the iteration's [P, 2(d), 2(hh), SB]
                        # tile; head hh's k-tile pair is the strided slice
                        # [:, :, hh, :]
                        h = 2 * hc + hh

                        def pav():
                            if bfslab:
                                for d_ in range(2):
                                    t = tb + d_
                                    ql = qlo(t)
                                    nc.tensor.matmul(
                                        outp[hh][:, ql:],
                                        lhsT=vbf_sb[:, t, h, :],
                                        rhs=expT_t[:, d_, hh, ql:],
                                        start=(t == 0), stop=(t == n_kt - 1))
                            elif (causal and tb >= 4 * j) or \
                                    mask_mode == "generic":
                                # masked pairs run as 2 plain fp8 matmuls:
                                # their [d_, hh, qp:] rhs reads structurally
                                # match the strip/mask tensor_mul writes, so
                                # the dependency tracker sees them
                                for d_ in range(2):
                                    nc.tensor.matmul(
                                        outp[hh][:, qp:],
                                        lhsT=v_sb[:, tb // 2, h, d_, :],
                                        rhs=expT_t[:, d_, hh, qp:],
                                        start=(tb == 0 and d_ == 0),
                                        stop=(d_ == 1 and tb == n_kt - 2))
                            else:
                                # fused P@V: one DoubleRow matmul per k-tile
                                # pair (2 fp8 weights/cell; virtual K=256)
                                nc.tensor.matmul(
                                    outp[hh][:, qp:],
                                    lhsT=v_sb[:, tb // 2, h, :, :],
                                    rhs=expT_t[:, :, hh, qp:],
                                    start=(tb == 0), stop=(tb == n_kt - 2),
                                    perf_mode=mybir.MatmulPerfMode.DoubleRow)
                        return pav

                    # d-major iteration: one [P,2,SB] PSUM tile per k-tile of
                    # the pair, each holding BOTH heads' scores (bank=hh).
                    # The 2 MMs of a d become READY together (single WAR edge
                    # on that d's exp), so the scheduler keeps them adjacent
                    # and — being different row groups (h0/h64) — they run
                    # CONCURRENT in the array.  (With per-hh tiles, readiness
                    # was staggered by the previous iteration's exps and the
                    # scheduler regrouped the MMs per row group — no
                    # concurrency.  With one merged 4-bank tile, scores(i+1)
                    # had to wait the FULL exp(i) — 1.5us PE stalls and HAM
                    # re-throttle.)  exp runs per d over both heads (N=1024);
                    # expT is [P, 2(d), 2(hh), SB] so the DR P@V rhs is the
                    # clean strided slice [:, :, hh, qp:].
                    for tb in range(0, n_kt, 2):
                        myfill = fillers.pop(it, ())
                        it += 1
                        qb = qlo(tb)
                        qp = qb
                        mloads = []
                        # both tiles write from the PAIR base qb: exp reads
                        # [qb:], and a never-written stale PSUM value there
                        # can exp() to inf -> fp8 INF -> NaN via the
                        # strip-zero multiply.
                        spd = [sps.tile([P, 2, SB], F32, tag="sps", name="sp")
                               for _ in range(2)]
                        expT4 = exp_pool.tile([P, 2, 2, SB], edt, tag=etag,
                                              name="expT")
                        for d_ in range(2):
                            t = tb + d_
                            for hh in range(2):
                                hp = DH * hh
                                nc.tensor.matmul(
                                    spd[d_][:, hh, qb:],
                                    lhsT=kT_sb[hp:hp + DH, hc,
                                               t * P:(t + 1) * P],
                                    rhs=qT_sb[hp:hp + DH, hc,
                                              j * SB + qb:(j + 1) * SB],
                                    start=True, stop=True,
                                    tile_position=(hp, 0))
                        # bias -3.5: keeps exp within fp8e4 range (softmax
                        # is shift-invariant; numerator and denominator
                        # share the e^-3.5 factor)
                        for d_ in range(2):
                            nc.scalar.activation(
                                expT4[:, d_, :, qb:],
                                spd[d_][:, 0:2, qb:], AFT.Exp, scale=SCALE,
                                bias=expbias_sb[:])
                        if dump and j == 3 and hc == 0 and not bfslab:
                            for hh in range(2):
                                nc.sync.dma_start(
                                    expT_o[:, hh, tb // 2, :, :],
                                    expT4[:, :, hh, :])
                        # pending P@Vs go BEFORE the fillers: they are always
                        # data-ready (their v tiles came from earlier
                        # iterations), while a filler stuck on a DMA arrival
                        # would block them in the PE FIFO and idle the array
                        # long enough to re-throttle the clock
                        for hh in range(2):
                            if len(pend[hh]) >= 2:
                                pend[hh].pop(0)()
                        # fillers after this pair's scores (so they are not
                        # queued ahead of them) but before the P@V that may
                        # consume their output
                        for f in myfill:
                            f()
                        for d_ in range(2):
                            t = tb + d_
                            if mask_mode == "generic":
                                m_sb = stage.tile([P, SB], BF16, tag="msk",
                                                  name="m_sb")
                                nc.gpsimd.dma_start(
                                    m_sb[:],
                                    maskT_d[t * P:(t + 1) * P,
                                            j * SB:(j + 1) * SB])
                                mloads.append(m_sb)
                            if mask_mode == "generic":
                                for hh in range(2):
                                    nc.vector.tensor_mul(
                                        expT4[:, d_, hh, :],
                                        expT4[:, d_, hh, :],
                                        mloads[d_][:])
                            elif causal and t >= 4 * j:
                                # mask relative to the PAIR's q window:
                                # d_=0: triangle on 128 cols; d_=1: zero
                                # prefix + triangle over 256 cols; one mul
                                # covers BOTH heads' banks of this d
                                w = min((d_ + 1) * P, SB - qp)
                                for hh in range(2):
                                    nc.vector.tensor_mul(
                                        expT4[:, d_, hh, qp:qp + w],
                                        expT4[:, d_, hh, qp:qp + w],
                                        strips_sb[:, d_, 0:w])
                        for hh in range(2):
                            pend[hh].append(make_pav(tb, qp, hh, expT4))
                    for hh in range(2):
                        for f in pend[hh]:
                            f()
                    if dump and j == 3 and hc == 0:
                        for hh in range(2):
                            oc = stage.tile([P, SB], F32, tag="recip",
                                            name="oc_dump")
                            nc.vector.tensor_copy(oc[:], outp[hh][:])
                            nc.sync.dma_start(outp_o[:, hh, 0, :], oc[:])
                    # normalize: rows 64:128 of outp hold the denominator.
                    # reciprocal_approx_fast mis-reads when in/out partition
                    # bases differ, so run it over all 128 rows (rows 0:64
                    # are recip of the unnormalized output — unused).
                    for hh in range(2):
                        hp = DH * hh
                        recip = stage.tile([P, SB], F32, tag="recip",
                                           name="recip")
                        nc.vector.reciprocal_approx_fast(recip[:], outp[hh][:])
                        nc.vector.tensor_mul(
                            headsT_sb[hp:hp + DH, hc, j * SB:(j + 1) * SB],
                            outp[hh][0:DH, :], recip[DH:P, :])
                for k in sorted(fillers):
                    for f in fillers[k]:
                        f()

            def F(fn, *a):
                return lambda: fn(*a)

            # ---- schedule: slabs in descending work order; projection and
            # output-projection units woven into attention iterations so the
            # PE stays dense while ACT (exp) is the bottleneck.
            # PE warmup: dummy matmuls run while the first DMAs land and
            # flip the HAM clock gate to 2.4 GHz before real work arrives
            def ka(n, w=P):
                # keep-alive matmuls: no data deps, so they run the moment
                # the PE FIFO reaches them.  A short (N=128) pulse is enough
                # to break the HAM idle window during a DMA-arrival stall;
                # only the initial warm-up burst needs sustained busy (w=SB).
                for i in range(n):
                    wps = pp.tile([P, SB], F32, tag="pp", name="warm_ps")
                    nc.tensor.matmul(wps[:, 0:w], lhsT=wsrc[:, 0:P],
                                     rhs=wsrc[:, 0:w], start=True, stop=True)

            ka(12, w=SB)
            proj_unit("k", 0, 0)
            # bridge the xq-b3 DMA wait (~2.5us) so the HAM clock gate never
            # re-throttles between the first projections and slab 3
            ka(14, w=SB)
            proj_unit("q", 0, 3)
            # Filler placement rule: each unit is emitted at (or before) the
            # iteration whose instructions first consume its output —
            # scores(hc, tb) need kT/qT of j-slab tb//4, P@V(hc0, tb) needs
            # v tiles tb..tb+1 (iteration index tb//2).
            attn_slab(3, {
                0: [F(ka, 1), F(vproj_unit, 0)],
                1: [F(ka, 1), F(vproj_unit, 2), F(proj_unit, "k", 0, 1),
                    F(proj_unit, "q", 1, 3)],
                2: [F(ka, 1), F(vproj_unit, 4)],
                3: [F(ka, 1), F(vproj_unit, 6), F(proj_unit, "k", 0, 2)],
                4: [F(ka, 1), F(vproj_unit, 8)],
                5: [F(ka, 1), F(vproj_unit, 10), F(proj_unit, "k", 0, 3),
                    F(proj_unit, "k", 1, 0)],
                6: [F(ka, 1), F(vproj_unit, 12)],
                7: [F(ka, 1), F(vproj_unit, 14)],
                9: [F(proj_unit, "k", 1, 1), F(proj_unit, "q", 0, 2)],
                11: [F(proj_unit, "k", 1, 2)],
                13: [F(proj_unit, "k", 1, 3)],
            })
            attn_slab(2, {
                0: [F(oproj_unit, 12)],
                1: [F(oproj_unit, 13)],
                2: [F(oproj_unit, 14), F(proj_unit, "q", 1, 2)],
                3: [F(oproj_unit, 15)],
                6: [F(proj_unit, "q", 0, 1)],
                8: [F(proj_unit, "q", 1, 1)],
            })
            attn_slab(1, {
                0: [F(oproj_unit, 8)], 1: [F(oproj_unit, 9)],
                2: [F(oproj_unit, 10)], 3: [F(oproj_unit, 11)],
                5: [F(proj_unit, "q", 0, 0)],
                6: [F(proj_unit, "q", 1, 0)],
            })
            attn_slab(0, {
                0: [F(ka, 1), F(oproj_unit, 4)],
                1: [F(ka, 1), F(oproj_unit, 5)],
                2: [F(ka, 1), F(oproj_unit, 6)],
                3: [F(ka, 1), F(oproj_unit, 7)],
            })
            # bridge the slab-0 hc1 normalize chain (~2.2us of DVE with no PE
            # work ready) so the final output projections run at full clock
            ka(6, w=SB)
            for st in range(4):
                ka(2, w=SB)
                oproj_unit(st, pool=sps)
            ka(2)

            if dump:
                nc.scalar.dma_start(qT_o, qT_sb[:])
                nc.scalar.dma_start(kT_o, kT_sb[:])
                nc.scalar.dma_start(v_o, v_sb[:])
                nc.scalar.dma_start(vbf_o, vbf_sb[:])
                nc.scalar.dma_start(heads_o, headsT_sb[:])
                nc.scalar.dma_start(strips_o, strips_sb[:])

    nc.compile()
    return nc


@functools.lru_cache(maxsize=4)
def _get(mask_mode: str):
    return _build(mask_mode)


def _bf16(a):
    return np.ascontiguousarray(a.astype(ml_dtypes.bfloat16))


def _detect_mask_mode(m):
    if (m == 1).all():
        return "none"
    idx = np.arange(m.shape[0])
    if np.array_equal(m != 0, idx[None, :] <= idx[:, None]):
        return "causal"
    return "generic"


def _strips():
    p = np.arange(P)[:, None]
    f = np.arange(SB)[None, :]
    s = np.stack([(p <= f - P * i) for i in range(SB // P)], axis=1)
    return np.ascontiguousarray(s.astype(ml_dtypes.bfloat16))


def prepare(query, key, value, mask, Wq, bq, Wk, bk, Wv, bv, Wo, bo):
    """Returns (mask_mode, in_maps) for run_bass_kernel_spmd."""
    query = np.asarray(query, dtype=np.float32)
    key = np.asarray(key, dtype=np.float32)
    value = np.asarray(value, dtype=np.float32)
    m2d = np.asarray(mask).reshape(np.asarray(mask).shape[-2:])
    mask_mode = _detect_mask_mode(m2d)

    def prep_x(x):    # [S, D] -> 512-blocked transposed [NSLAB, P, DK*SB] f32
        # xT[b, p, o*SB + c] = x[SB*b + c, P*o + p]
        xt = np.asarray(x, np.float32).T.reshape(DK, P, NSLAB, SB)
        return np.ascontiguousarray(
            xt.transpose(2, 1, 0, 3).reshape(NSLAB, P, DK * SB))

    def _fp8(a):
        return np.ascontiguousarray(a.astype(ml_dtypes.float8_e4m3))

    xq = [_bf16(prep_x(query[b])) for b in range(B)]
    xk = [_bf16(prep_x(key[b])) for b in range(B)]
    xvf = [prep_x(value[b]) for b in range(B)]
    xv0 = [_bf16(xvf[b][0]) for b in range(B)]
    xv8 = [_fp8(xvf[b][1:]) for b in range(B)]

    def prep_w(W, g):     # rows [256g, 256g+256) of W, transposed -> [128, 8, 256]
        sl = np.asarray(W, np.float32)[g * C:(g + 1) * C, :].T
        return _bf16(sl.reshape(DK, P, C).transpose(1, 0, 2))

    def prep_wo(g):       # Wo[:, 256g:256g+256].T -> [128, 2, 1024]
        sl = np.asarray(Wo, np.float32)[:, g * C:(g + 1) * C].T
        return _bf16(sl.reshape(C // P, P, D).transpose(1, 0, 2))

    def prep_b(b_, g):
        sl = np.asarray(b_, np.float32)[g * C:(g + 1) * C]
        return np.ascontiguousarray(sl.reshape(C // P, P).T)

    def prep_bvb(g):
        sl = np.asarray(bv, np.float32)[g * C:(g + 1) * C]
        return np.ascontiguousarray(np.broadcast_to(sl[None, :], (P, C)))

    strips = _strips()
    maskT = _bf16(m2d.T.astype(np.float32)) if mask_mode == "generic" else None

    in_maps = []
    for c in range(NCORES):
        b, g = c // GROUPS, c % GROUPS
        cb = np.concatenate([
            prep_w(Wq, g).reshape(P, 2048), prep_w(Wk, g).reshape(P, 2048),
            prep_w(Wv, g).reshape(P, 2048), prep_wo(g).reshape(P, 2048),
            strips.reshape(P, 2048)], axis=1)
        cf = np.concatenate([
            prep_b(bq, g), prep_b(bk, g), prep_bvb(g)], axis=1)
        def prep_w8(W):
            return (np.asarray(W, np.float32)[g * C:(g + 1) * C, :].T
                    .reshape(DK, P, C).transpose(1, 0, 2).reshape(P, DK * C))

        im = dict(xq=xq[b], xk=xk[b], xv0=xv0[b], xv8=xv8[b],
                  cb=np.ascontiguousarray(cb),
                  cb8=_fp8(prep_w8(Wv)),
                  cf=np.ascontiguousarray(cf.astype(np.float32)))
        if maskT is not None:
            im["maskT"] = maskT
        in_maps.append(im)

    return mask_mode, in_maps


def kernel(query, key, value, mask, Wq, bq, Wk, bk, Wv, bv, Wo, bo):
    mask_mode, in_maps = prepare(query, key, value, mask, Wq, bq, Wk, bk,
                                 Wv, bv, Wo, bo)
    nc = _get(mask_mode)
    res = run_bass_kernel_spmd(nc, in_maps, list(range(NCORES)))
    partials = np.stack([res.results[c]["o"].astype(np.float32)
                         for c in range(NCORES)])
    out = partials.reshape(B, GROUPS, S, D).sum(axis=1)
    out = out + np.asarray(bo, np.float32)[None, None, :]
    return out.astype(np.float32)

